# revision 35
# baseline (speedup 1.0000x reference)
"""Cross-attention kernel for Trainium2, 8 NeuronCores (fp8 DoubleRow rev).

Reference computation (B=4, S=2048, C=1024, E=1024, D=768, H=16, hd=64):
    q = x @ q_w + q_b                 # [B,S,E]
    k = context @ k_w + k_b           # [B,C,E]
    v = context @ v_w + v_b           # [B,C,E]
    attn = softmax(q.k^T / sqrt(hd))  # per head
    out = (attn @ v) @ o_w + o_b      # [B,S,E]

Sharding: 8 cores = 4 batches x 2 head-groups (8 heads = 512 embed cols each).
Each core computes the full attention for its (batch, head-group) and a
partial out-projection; the host sums the two head-group partials per batch
(the "all-reduce") and adds o_b.

Device schedule (v3, fp8): matmul cost on the PE is (out free size) x
(cycles/row); fp8e4 with MatmulPerfMode.DoubleRow runs 0.5 cycles/row with a
256-deep contraction (2 fp8 weights per PE cell), 4x the fp16 MAC rate.
The error gate (2e-2 rel) rules out fp8 on the value path (V, P, attn-out,
o_w each land ~2.4% e4m3 quantization 1:1 in the final output), so fp8 is
applied only where the error lands as an *absolute* score perturbation:

  - scores (q.k): q and kt stored fp8e4 (sqrt(softmax scale) folded into
    each to stay clear of fp8 subnormals).  DoubleRow with the contraction
    zero-padded 64->128: kt tiles are [128, 2, C] with sub1 = 0 (0 x finite
    junk = 0; NaN would poison, so the q sub1 halves are zeroed too).
    131072 -> 65536 PE cycles.
  - q/k/v projections: both operands come from DRAM, so the host ships
    fp8 value+residual pairs (a = a8 + aR) and the kernel accumulates
    a8@w8 + a8@wR + aR@w8 in one PSUM group - fp16-level accuracy at
    3 x 0.25 = 0.75x the fp16 cost.  Weights are pre-scaled x256 (values
    would sit in fp8 subnormal range); the epilogue tensor_scalar folds
    the 1/256 into its existing multiply, and for V the x256 rides into
    the fp16 V tiles and is cancelled by scaling the softmax-denominator
    ones column x256 (the reciprocal then yields 1/(256*sum_p)).

attn-V keeps the fp16 probabilities-stationary form with the [V_h | c]
65-wide moving operand; out-projection keeps fp16 with PE transposes.

Engine budget per core (cost model): ACT exp 128x[128,1024] ~= 133us is the
wall; PE ~= 122us (was 161), DVE ~= 80us (normalize batched via a stride-0
broadcast tensor_tensor, transpose copies merged to [128,512]).
"""

import sys

sys.path.insert(0, "/opt/trn_rl_repo")

import numpy as np
import ml_dtypes

F8NP = ml_dtypes.float8_e4m3

B, S, E, C, D = 4, 2048, 1024, 1024, 768
H, HD = 16, 64
EL = E // 2          # embed columns per head-group (8 heads)
N_CORES = 8
NS = S // 512        # s-tiles of 512
NEP = E // 256       # Q-proj contraction double-chunks (4)
NDP = D // 256       # K/V-proj contraction double-chunks (3)
CC = C // 128        # c chunks of 128
HP = EL // 128       # head pairs per core (4)
WS = 256.0           # fp8 weight pre-scale (q/k/v projection weights)

_built = None
_last_results = None


def _build(reps=1, nop_us=0):
    import concourse.bacc as bacc
    import concourse.mybir as mybir
    from concourse.tile import TileContext

    F32 = mybir.dt.float32
    F16 = mybir.dt.float16
    F8 = mybir.dt.float8e4
    Exp = mybir.ActivationFunctionType.Exp
    DR = mybir.MatmulPerfMode.DoubleRow

    nc = bacc.Bacc(None, target_bir_lowering=False)

    x8 = nc.declare_dram_parameter("x8", [E, S], F8, isOutput=False)
    xr = nc.declare_dram_parameter("xr", [E, S], F8, isOutput=False)
    ctxp = nc.declare_dram_parameter("ctxp", [D, 2 * C], F8, isOutput=False)
    qwp = nc.declare_dram_parameter("qwp", [128, 2 * NEP * 2 * EL], F8,
                                    isOutput=False)
    kwp = nc.declare_dram_parameter("kwp", [128, 2 * NDP * 2 * EL], F8,
                                    isOutput=False)
    vw8 = nc.declare_dram_parameter("vw8", [128, NDP * 2 * EL], F8,
                                    isOutput=False)
    vwr = nc.declare_dram_parameter("vwr", [128, NDP * 2 * EL], F8,
                                    isOutput=False)
    ow = nc.declare_dram_parameter("ow", [EL, E], F16, isOutput=False)
    kqb = nc.declare_dram_parameter("kqb", [EL, 2], F32, isOutput=False)
    vbones = nc.declare_dram_parameter("vbones", [1, EL + 128], F16,
                                       isOutput=False)
    ident = nc.declare_dram_parameter("ident", [128, 128], F16, isOutput=False)
    out = nc.declare_dram_parameter("out", [S, E], F16, isOutput=True)

    with TileContext(nc) as tc:
        with (
            tc.tile_pool(name="wpool", bufs=1) as wpool,
            tc.tile_pool(name="dpool", bufs=1) as dpool,
            tc.tile_pool(name="xpool", bufs=2) as xpool,
            tc.tile_pool(name="ptpool", bufs=28) as ptpool,
            tc.tile_pool(name="otpool", bufs=4) as otpool,
            tc.tile_pool(name="ttpool", bufs=12) as ttpool,
            tc.tile_pool(name="spool", bufs=2) as spool,
            tc.tile_pool(name="opool", bufs=2) as opool,
            tc.tile_pool(name="pspool", bufs=1, space="PSUM") as pspool,
        ):
          for _rep in range(reps):
            # ---- weight / context tiles (fp8 value + residual merged into
            # single params: each dma_start burns a ~630ns global issue slot,
            # so the value/residual pair rides one DMA) -------------------
            qwp_t = wpool.tile([128, 2 * NEP * 2 * EL], F8, name="qwp_t")
            qwp_v = qwp_t.rearrange("p (t j i m) -> p t j i m", t=2, j=NEP, i=2)
            qw8_v, qwr_v = qwp_v[:, 0], qwp_v[:, 1]
            kwp_t = wpool.tile([128, 2 * NDP * 2 * EL], F8, name="kwp_t")
            kwp_v = kwp_t.rearrange("p (t j i m) -> p t j i m", t=2, j=NDP, i=2)
            kw8_v, kwr_v = kwp_v[:, 0], kwp_v[:, 1]
            vw8_t = wpool.tile([128, NDP * 2 * EL], F8, name="vw8_t")
            vw8_v = vw8_t.rearrange("p (j i m) -> p j i m", j=NDP, i=2)
            vwr_t = wpool.tile([128, NDP * 2 * EL], F8, name="vwr_t")
            vwr_v = vwr_t.rearrange("p (j i m) -> p j i m", j=NDP, i=2)
            # ctx layout [p, col-half, d-chunk, value/resid, 512] so the
            # half-column DMAs balance to 3 dims
            ctxp_t = dpool.tile([128, 2 * 6 * 2 * 512], F8, name="ctxp_t")
            ctxp_v = ctxp_t.rearrange("p (h d t c) -> p h d t c",
                                      h=2, d=6, t=2)
            ctx8h = [ctxp_v[:, 0, :, 0, :], ctxp_v[:, 1, :, 0, :]]
            cxrh = [ctxp_v[:, 0, :, 1, :], ctxp_v[:, 1, :, 1, :]]

            # All DMA transfers serialize on one global lane in the cost
            # model (each dma_start also burns a ~630ns issue slot), so the
            # prologue issue order IS the arrival order.  Order the lane so
            # each consumer's last dependency lands just before it runs:
            #   kwp, x8, qwp[m0], kqb, ctxp[cols 0-511], xr, vw8, vbones,
            #   ctxp[cols 512-1023], vwr, ident, qwp[m1-3], ow
            kqb_t = wpool.tile([128, 2 * HP], F32, name="kqb_t")
            kb_sb = [kqb_t[:, 2 * m:2 * m + 1] for m in range(HP)]
            qb_sb = [kqb_t[:, 2 * m + 1:2 * m + 2] for m in range(HP)]
            qwp_p = qwp.rearrange("p (t j i m) -> p t j i m", t=2, j=NEP, i=2)
            ctxp_p = ctxp.rearrange("(k p) (h t c) -> p k h t c",
                                    p=128, h=2, t=2)
            nc.sync.dma_start(out=kwp_t[:], in_=kwp[:])
            nc.sync.dma_start(out=qwp_v[:, :, :, :, 0:128],
                              in_=qwp_p[:, :, :, :, 0:128])

            # ---- K^T tiles: [hd-pair 128, sub 2, C] fp8, sub1 = 0 ---------
            kt_sb = []
            kt_v = []
            for m in range(HP):
                t = dpool.tile([128, 2 * C], F8, name=f"kt{m}")
                kt_sb.append(t)
                kt_v.append(t.rearrange("p (i c) -> p i c", i=2))

            # ---- static Q^T tiles: [128, parity 2, sub 2, 512] fp8 --------
            q8_t = []
            q8_v = []
            for m in range(HP):
                t = wpool.tile([128, 2 * 2 * 512], F8, name=f"q8_{m}")
                q8_t.append(t)
                q8_v.append(t.rearrange("p (a i s) -> p a i s", a=2, i=2))

            def zmem(m):
                return []   # kt/q8 sub1 now carry kR / duplicated q8

            # ---- V tiles: [C rows, 8 heads x 65] fp16 (values x WS) -------
            v_sb = []
            for mc in range(CC):
                t = dpool.tile([128, 8 * 65], F16, name=f"v{mc}")
                v_sb.append(t)

            # ---- projection matmul thunks (3-pass fp8 DoubleRow) ----------
            def kgroup_thunks(pairs):
                """Per-matmul thunks for K-proj groups (hp, chalf).  The
                epilogue splits: the PSUM-reading descale runs with the
                group (freeing the acc bank for the next group quickly);
                the fp8 quantize + DoubleRow-residual writes into kt are
                appended as separate thunks so the DVE drain doesn't stall
                the PE's acc-slot pipeline."""
                state = {}

                def f(g, i):
                    hp, chalf = pairs[g]
                    if i == 0:
                        state[g] = pspool.tile([128, 512], F32, name="acc_ps",
                                               tag="acc", bufs=2)
                    ps = state[g]
                    pi, j = i // NDP, i % NDP
                    st_v = [kw8_v, kwr_v, kw8_v][pi]
                    mv_v = [ctx8h, ctx8h, cxrh][pi][chalf]
                    nc.tensor.matmul(
                        ps[:],
                        st_v[:, j, :, hp * 128:(hp + 1) * 128],
                        mv_v[:, 2 * j:2 * j + 2, :],
                        start=(i == 0), stop=(i == 8), perf_mode=DR,
                    )
                    if i == 8:
                        kv = spool.tile([128, 512], F32, name="ksc",
                                        tag="ksc", bufs=4)
                        state[(g, 'kv')] = kv
                        nc.vector.tensor_scalar(
                            kv[:], ps[:], 1.0 / WS, kb_sb[hp],
                            mybir.AluOpType.mult, mybir.AluOpType.add,
                        )

                def fin(g, _):
                    hp, chalf = pairs[g]
                    cs = slice(chalf * 512, (chalf + 1) * 512)
                    kv = state.pop((g, 'kv'))
                    nc.vector.tensor_copy(kt_v[hp][:, 0, cs], kv[:])
                    nc.vector.tensor_tensor(
                        kt_v[hp][:, 1, cs], kv[:], kt_v[hp][:, 0, cs],
                        mybir.AluOpType.subtract,
                    )

                out = []
                for g in range(len(pairs)):
                    out.extend((f, g, i) for i in range(9))
                    out.append((fin, g, 0))
                return out

            def vproj_thunks():
                state = {}

                def f(mc, i):
                    if i == 0:
                        state[mc] = pspool.tile([128, 512], F32,
                                                name="acc_ps", tag="acc",
                                                bufs=2)
                    ps = state[mc]
                    pi, j = i // NDP, i % NDP
                    st_v = [ctx8h, ctx8h, cxrh][pi][mc // 4]
                    mv_v = [vw8_v, vwr_v, vw8_v][pi]
                    nc.tensor.matmul(
                        ps[:],
                        st_v[:, 2 * j:2 * j + 2,
                             (mc % 4) * 128:(mc % 4 + 1) * 128],
                        mv_v[:, j, :, :],
                        start=(i == 0), stop=(i == 8), perf_mode=DR,
                    )
                    if i == 8:
                        vv = v_sb[mc].rearrange("p (h u) -> p h u", u=65)
                        nc.vector.tensor_add(
                            vv[:, :, 0:64],
                            ps.rearrange("p (h u) -> p h u", u=64),
                            vb_bc.rearrange("p (h u) -> p h u", u=64),
                        )
                        nc.vector.tensor_scalar(
                            vv[:, :, 64:65],
                            vb_bc[:, 0:8].rearrange("p (h u) -> p h u", u=1),
                            0.0, WS,
                            mybir.AluOpType.mult, mybir.AluOpType.add,
                        )  # denom column = WS (folds V's x WS into recip)

                return [(f, mc, i) for mc in range(CC) for i in range(9)]

            # ---- pipelined main loop over s-tiles of 512 ------------------
            xts_all = {}
            tts_all = {}

            def load_x(n, e8=None, er=None):
                t8 = xpool.tile([128, 8 * 512], F8, name="x8t", tag="x8t")
                tr = xpool.tile([128, 8 * 512], F8, name="xrt", tag="xrt")
                (e8 or nc.sync).dma_start(
                    out=t8.rearrange("p (c w) -> p c w", w=512),
                    in_=x8[:, n * 512:(n + 1) * 512]
                    .rearrange("(c p) w -> p c w", p=128),
                )
                (er or nc.sync).dma_start(
                    out=tr.rearrange("p (c w) -> p c w", w=512),
                    in_=xr[:, n * 512:(n + 1) * 512]
                    .rearrange("(c p) w -> p c w", p=128),
                )
                xts_all[n] = (t8.rearrange("p (c w) -> p c w", w=512),
                              tr.rearrange("p (c w) -> p c w", w=512))

            def qproj_thunks(n):
                """48 DR-matmul thunks computing q8 (fp8) for s-tile n."""
                state = {}
                thunks = []
                par = n % 2

                def f(m, idx):
                    if idx == 0:
                        state[m] = pspool.tile(
                            [128, 512], F32, name="acc_ps", tag="acc", bufs=2)
                    ps = state[m]
                    pi, j = idx // NEP, idx % NEP
                    x8v, xrv = xts_all[n]
                    st_v = [qw8_v, qwr_v, qw8_v][pi]
                    mv = [x8v, x8v, xrv][pi]
                    nc.tensor.matmul(
                        ps[:],
                        st_v[:, j, :, m * 128:(m + 1) * 128],
                        mv[:, 2 * j:2 * j + 2, :],
                        start=(idx == 0), stop=(idx == 3 * NEP - 1),
                        perf_mode=DR,
                    )
                    if idx == 3 * NEP - 1:
                        nc.vector.tensor_scalar(
                            q8_v[m][:, par, 0, :], ps[:],
                            1.0 / WS, qb_sb[m],
                            mybir.AluOpType.mult, mybir.AluOpType.add,
                        )
                        nc.vector.tensor_copy(q8_v[m][:, par, 1, :],
                                              q8_v[m][:, par, 0, :])

                for m in range(HP):
                    for idx in range(3 * NEP):
                        thunks.append((f, m, idx))
                return thunks

            def outproj_thunks(n, copy_on_act=False):
                """32 matmul thunks for the out-projection of s-tile n (fp16).

                Stationary = transposed normalized attention tile
                tts_all[n][hp] slice ([128 hd-pair rows, 128 s cols], fp16);
                moving = ow chunk [128, 512].  The epilogue instance runs its
                PSUM->SBUF copies on the ACT engine (idle after the final
                exp) to keep the tail chain off the DVE."""
                state = {}
                thunks = []

                def f(ss, ne, hp):
                    if hp == 0:
                        state[(ss, ne)] = pspool.tile(
                            [128, 512], F32, name="acc_ps", tag="acc", bufs=2)
                        if ne == 0:
                            state[ss] = opool.tile(
                                [128, 1024], F16, name="o_sb", tag="o")
                    ps = state[(ss, ne)]
                    nc.tensor.matmul(
                        ps[:],
                        tts_all[n][hp][:, ss * 128:(ss + 1) * 128],
                        ow_sb[hp][:, ne * 512:(ne + 1) * 512],
                        start=(hp == 0), stop=(hp == HP - 1),
                    )
                    if hp == HP - 1:
                        o_sb = state[ss]
                        if copy_on_act:
                            nc.scalar.activation(
                                o_sb[:, ne * 512:(ne + 1) * 512], ps[:],
                                mybir.ActivationFunctionType.Copy)
                        else:
                            nc.vector.tensor_copy(
                                o_sb[:, ne * 512:(ne + 1) * 512], ps[:])
                        if ne == 1:   # one merged store per s-chunk row
                            nc.sync.dma_start(
                                out=out[n * 512 + ss * 128:
                                        n * 512 + (ss + 1) * 128, :],
                                in_=o_sb[:],
                            )

                for ss in range(4):
                    for ne in range(2):
                        for hp in range(HP):
                            thunks.append((f, ss, ne, hp))
                return thunks

            def run_thunks(ts):
                for f, *args in ts:
                    f(*args)

            # prologue: x(0)/qw on the DVE and Pool queues, then the zero
            # memsets for hp0; K-proj hp0 + Q-proj(0) m=0 run pre-loop so the
            # first exp fires ~6us in; everything else is paced into tile 0.
            nc.sync.dma_start(
                out=kqb_t.rearrange("p (c w) -> p c w", w=2),
                in_=kqb.rearrange("(c p) w -> p c w", p=128),
            )
            x8t0 = xpool.tile([128, 8 * 512], F8, name="x8t", tag="x8t")
            nc.sync.dma_start(
                out=x8t0.rearrange("p (c w) -> p c w", w=512),
                in_=x8[:, 0:512].rearrange("(c p) w -> p c w", p=128),
            )
            nc.sync.dma_start(out=ctxp_v[:, 0], in_=ctxp_p[:, :, 0])
            xrt0 = xpool.tile([128, 8 * 512], F8, name="xrt", tag="xrt")
            nc.sync.dma_start(
                out=xrt0.rearrange("p (c w) -> p c w", w=512),
                in_=xr[:, 0:512].rearrange("(c p) w -> p c w", p=128),
            )
            xts_all[0] = (x8t0.rearrange("p (c w) -> p c w", w=512),
                          xrt0.rearrange("p (c w) -> p c w", w=512))
            nc.sync.dma_start(out=ctxp_v[:, 1], in_=ctxp_p[:, :, 1])
            nc.sync.dma_start(out=vw8_t[:], in_=vw8[:])
            vbo_t = wpool.tile([1, EL + 128], F16, name="vbo_t")
            nc.sync.dma_start(out=vbo_t[:], in_=vbones[:])
            vb_sb = vbo_t[:, 0:EL]
            ones_sb = vbo_t[:, EL:EL + 128]
            nc.sync.dma_start(out=vwr_t[:], in_=vwr[:])
            ident_sb = wpool.tile([128, 128], F16, name="ident_sb")
            nc.sync.dma_start(out=ident_sb[:], in_=ident[:])
            vb_bc = wpool.tile([128, EL], F32, name="vb_bc")
            nc.sync.dma_start(out=qwp_v[:, :, :, :, 128:512],
                              in_=qwp_p[:, :, :, :, 128:512])
            ow_all = wpool.tile([128, HP * E], F16, name="ow_all")
            ow_sb = [ow_all[:, k * E:(k + 1) * E] for k in range(HP)]
            nc.sync.dma_start(
                out=ow_all.rearrange("p (c w) -> p c w", w=E),
                in_=ow.rearrange("(c p) w -> p c w", p=128),
            )
            qp0 = qproj_thunks(0)
            run_thunks(qp0[:2 * NEP])         # m=0 passes A+B
            run_thunks(kgroup_thunks([(0, 0)]))
            run_thunks(qp0[2 * NEP:3 * NEP])  # m=0 pass C + epilogue
            # vb broadcast for the V epilogue
            vb_ps = pspool.tile([128, 512], F32, name="acc_ps", tag="acc",
                                bufs=2)
            nc.tensor.matmul(vb_ps[:], ones_sb[0:1, :], vb_sb[:],
                             start=True, stop=True)
            nc.vector.tensor_copy(vb_bc[:], vb_ps[:])

            deferred_q = {}
            for n in range(NS):
                if n + 1 < NS:
                    load_x(n + 1)
                # anchors[i] = (step, bg-index that must be emitted by that
                # step).  Step map per tile: stage s scores = 4 steps, attn-V
                # = 4, transposes = 1; scores at stage 2m start at step
                # 12+17*(m-1) and need kt[m]+q8[m]; attn-V(0) (step 8) needs
                # the V tiles; stage-0 cpair2 (step 2) needs kt[0] cols
                # 512-1023.  inject() paces linearly between anchors.
                # attn-V pipeline depth: tile 0 runs 5 stages behind (the
                # K/V/Q-proj prologue work must fit before the V deadline);
                # later tiles 3 (keeps the last tile's tail short).
                # Step maps: scores(2m) start / attn-V(0) emission step.
                depth = 5 if n == 0 else (2 if n == NS - 1 else 3)
                SCST = {5: [0, 8, 16, 28], 3: [0, 8, 20, 37],
                        2: [0, 8, 25, 42]}[depth]
                AV0 = {5: 24, 3: 16, 2: 12}[depth]
                anchors = []
                bg = []
                if n == 0:
                    bg += kgroup_thunks([(0, 1)])
                    anchors.append((2, len(bg)))
                    bg += kgroup_thunks([(1, 0), (1, 1)])
                    bg += qp0[3 * NEP:2 * 3 * NEP]
                    anchors.append((SCST[1], len(bg)))
                    bg += kgroup_thunks([(2, 0), (2, 1)])
                    bg += qp0[2 * 3 * NEP:3 * 3 * NEP]
                    anchors.append((SCST[2], len(bg)))
                    bg += vproj_thunks()
                    anchors.append((AV0, len(bg)))
                    bg += kgroup_thunks([(3, 0), (3, 1)])
                    bg += qp0[3 * 3 * NEP:4 * 3 * NEP]
                    anchors.append((SCST[3], len(bg)))
                else:
                    dq = deferred_q.pop(n)
                    for m in range(1, HP):
                        bg += dq[(m - 1) * 3 * NEP:m * 3 * NEP]
                        anchors.append((SCST[m], len(bg)))
                if n + 1 < NS:
                    qp_next = qproj_thunks(n + 1)
                    bg += qp_next[:3 * NEP]          # m=0 in this tile
                    deferred_q[n + 1] = qp_next[3 * NEP:]
                if n >= 1:
                    bg += outproj_thunks(n - 1)
                anchors.append((62, len(bg)))

                tts_all[n] = [None] * HP
                par = n % 2
                n_steps = HP * (2 * 8 + 1)      # (hp, h2, 4 sc + 4 av) + tr
                step = 0
                bg_done = 0
                prev_anchor = [0, 0]

                def inject():
                    nonlocal bg_done
                    while anchors and step >= anchors[0][0]:
                        prev_anchor[:] = anchors.pop(0)
                    ns_, ni_ = anchors[0] if anchors else (n_steps, len(bg))
                    ps_, pi_ = prev_anchor
                    if step >= ns_:
                        target = ni_
                    else:
                        target = pi_ + (ni_ - pi_) * (step - ps_) // max(
                            1, ns_ - ps_)
                    target = max(target, pi_)
                    while bg_done < target:
                        fb, *args = bg[bg_done]
                        fb(*args)
                        bg_done += 1

                # software pipeline over stages s = hp*2 + h2: scores+exp of
                # stage s overlap the attn-V/normalize of stage s-1, so the
                # attn-V matmuls never wait on a just-issued exp.
                stage_p = {}
                ot_t = {}

                def emit_scores(s):
                    nonlocal step
                    hp, h2 = s // 2, s % 2
                    pts = []
                    for cpair in range(4):
                        sc = pspool.tile([128, 1024], F32, name="sc_ps",
                                         tag="sc", bufs=2)
                        for cc in range(2):
                            c = cpair * 2 + cc
                            nc.tensor.matmul(
                                sc[:, cc * 512:(cc + 1) * 512],
                                kt_v[hp][h2 * 64:(h2 + 1) * 64, :,
                                         c * 128:(c + 1) * 128],
                                q8_v[hp][h2 * 64:(h2 + 1) * 64, par, :, :],
                                start=True, stop=True, perf_mode=DR,
                            )
                        p = ptpool.tile([128, 1024], F16, name="pt",
                                        tag="pt")
                        nc.scalar.activation(p[:], sc[:], Exp,
                                             scale=1.0 / 16.0)
                        pts.append(p)
                        step += 1
                        inject()
                    stage_p[s] = pts

                def emit_av(s):
                    # one PSUM accumulation group per s-chunk, groups strictly
                    # sequential (a later group's start re-marks the whole
                    # bank pending-zero, so groups must not interleave)
                    nonlocal step
                    hp, h2 = s // 2, s % 2
                    if h2 == 0:
                        ot_t[hp] = otpool.tile([128, 512], F16, name="ot",
                                               tag="ot")
                    otv = ot_t[hp].rearrange("p (ss c) -> p ss c", c=128)
                    pts = stage_p.pop(s)
                    av = pspool.tile([128, 4 * 65], F32, name="av_ps",
                                     tag="av", bufs=1)
                    avv = av.rearrange("p (s u) -> p s u", u=65)
                    vv = [v_sb[c].rearrange("p (h u) -> p h u", u=65)
                          [:, hp * 2 + h2, :] for c in range(CC)]
                    for ss in range(4):
                        for c in range(CC):
                            nc.tensor.matmul(
                                avv[:, ss, :],
                                pts[c // 2][:, (c % 2) * 512 + ss * 128:
                                            (c % 2) * 512 + (ss + 1) * 128],
                                vv[c],
                                start=(c == 0), stop=(c == CC - 1),
                            )
                        step += 1
                        inject()
                    rs = spool.tile([128, 4], F32, name="rs", tag="rs")
                    with nc.allow_low_precision("softmax denom"):
                        nc.vector.reciprocal(
                            rs.rearrange("p (s u) -> p s u", u=1),
                            avv[:, :, 64:65])
                    # batched normalize: all 4 s-chunks in one tensor_tensor
                    # via a stride-0 broadcast of the per-chunk reciprocal
                    nc.vector.tensor_tensor(
                        otv[:, :, h2 * 64:(h2 + 1) * 64],
                        avv[:, :, 0:64],
                        rs[:].rearrange("p (s u) -> p s u", u=1)
                        .broadcast_to([128, 4, 64]),
                        mybir.AluOpType.mult,
                    )

                def emit_transposes(hp):
                    # transpose ot -> [128 hd-pair, 128 s] fp16 stationary
                    # for the out-projection; single merged PSUM->SBUF copy
                    nonlocal step
                    step += 1
                    inject()
                    otv = ot_t[hp].rearrange("p (ss c) -> p ss c", c=128)
                    trv = pspool.tile([128, 512], F16, name="tr_ps",
                                      tag="tr", bufs=1)
                    for ss in range(4):
                        nc.tensor.matmul(
                            trv[:, ss * 128:(ss + 1) * 128],
                            otv[:, ss, :], ident_sb[:],
                            is_transpose=True, start=True, stop=True,
                        )
                    tt = ttpool.tile([128, 512], F16, name="tt", tag="tt")
                    nc.vector.tensor_copy(tt[:], trv[:])
                    tts_all[n][hp] = tt

                for s in range(8):
                    emit_scores(s)
                    if s >= depth:
                        emit_av(s - depth)
                        if (s - depth) % 2 == 1:   # both h2 of a pair done
                            emit_transposes((s - depth) // 2)
                run_thunks(bg[bg_done:])
                for s2 in range(8 - depth, 8):
                    emit_av(s2)
                    if s2 % 2 == 1:
                        emit_transposes(s2 // 2)

            # epilogue: out-projection of the last s-tile
            run_thunks(outproj_thunks(NS - 1, copy_on_act=True))

          # timing aid: calibrated delay chain on the otherwise-idle gpsimd
          # engine; kernel exec time = max(real work, nop chain)
          if nop_us:
            NOP_CYC = 48000
            for _ in range(int(nop_us * 1200 / NOP_CYC)):
                nc.gpsimd.nop(cycle_cnt=NOP_CYC, nofuse=True)

    nc.finalize()
    return nc


def _fp8_pair(a):
    """fp8 value + residual pair such that a ~= a8 + ar (float32 in)."""
    a8 = a.astype(F8NP)
    ar = (a - a8.astype(np.float32)).astype(F8NP)
    return a8, ar


def _pack_w(w, npair):
    """[npair*256, EL] -> [128, npair, 2, EL] DoubleRow packing, flattened."""
    return np.ascontiguousarray(
        w.reshape(npair, 2, 128, EL).transpose(2, 0, 1, 3).reshape(128, -1))


def kernel(x, context, q_w, q_b, k_w, k_b, v_w, v_b, o_w, o_b):
    global _built, _last_results
    from concourse.bass_utils import run_bass_kernel_spmd

    if _built is None:
        _built = _build()
    nc = _built

    # sqrt(softmax scale) x4 per side: scores land x16 in PSUM (descaled in
    # the exp's scale arg); keeps fp8 q/k/kR values clear of subnormals
    rscale = np.float32(4.0 / np.sqrt(np.sqrt(HD)))
    x = np.asarray(x, np.float32)
    context = np.asarray(context, np.float32)
    x8s, xrs, c8s, crs = [], [], [], []
    for b in range(B):
        a8, ar = _fp8_pair(np.ascontiguousarray(x[b].T))
        x8s.append(a8)
        xrs.append(ar)
        a8, ar = _fp8_pair(np.ascontiguousarray(context[b].T))
        c8s.append(a8)
        crs.append(ar)

    in_maps = []
    for core in range(N_CORES):
        b, hg = core // 2, core % 2
        el = slice(hg * EL, (hg + 1) * EL)
        qw = np.asarray(q_w, np.float32)[:, el] * rscale * WS
        kw = np.asarray(k_w, np.float32)[:, el] * rscale * WS
        vw = np.asarray(v_w, np.float32)[:, el] * WS
        qw8, qwr = _fp8_pair(qw)
        kw8, kwr = _fp8_pair(kw)
        vw8, vwr = _fp8_pair(vw)
        in_maps.append({
            "x8": x8s[b], "xr": xrs[b],
            "ctxp": np.ascontiguousarray(
                np.stack([c8s[b], crs[b]], axis=1).reshape(D, 2, 2, 512)
                .transpose(0, 2, 1, 3).reshape(D, 2 * C)),
            "qwp": np.concatenate(
                [_pack_w(qw8, NEP), _pack_w(qwr, NEP)], axis=1),
            "kwp": np.concatenate(
                [_pack_w(kw8, NDP), _pack_w(kwr, NDP)], axis=1),
            "vw8": _pack_w(vw8, NDP), "vwr": _pack_w(vwr, NDP),
            "ow": np.ascontiguousarray(
                np.asarray(o_w, np.float32)[el, :]).astype(np.float16),
            "kqb": np.ascontiguousarray(np.stack(
                [np.asarray(k_b, np.float32)[el] * rscale,
                 np.asarray(q_b, np.float32)[el] * rscale], axis=1)),
            "vbones": np.ascontiguousarray(np.concatenate(
                [np.asarray(v_b, np.float32)[el] * WS,
                 np.ones(128, np.float32)])[None, :]).astype(np.float16),
            "ident": np.eye(128, dtype=np.float16),
        })

    res = run_bass_kernel_spmd(nc, in_maps, list(range(N_CORES)))
    _last_results = res

    ob = np.asarray(o_b, np.float32)
    full = np.empty((B, S, E), np.float32)
    for b in range(B):
        full[b] = (res.results[2 * b]["out"].astype(np.float32)
                   + res.results[2 * b + 1]["out"].astype(np.float32) + ob)
    return full


# revision 36
# speedup vs baseline: 1.0008x; 1.0008x over previous
"""Cross-attention kernel for Trainium2, 8 NeuronCores (fp8 DoubleRow rev).

Reference computation (B=4, S=2048, C=1024, E=1024, D=768, H=16, hd=64):
    q = x @ q_w + q_b                 # [B,S,E]
    k = context @ k_w + k_b           # [B,C,E]
    v = context @ v_w + v_b           # [B,C,E]
    attn = softmax(q.k^T / sqrt(hd))  # per head
    out = (attn @ v) @ o_w + o_b      # [B,S,E]

Sharding: 8 cores = 4 batches x 2 head-groups (8 heads = 512 embed cols each).
Each core computes the full attention for its (batch, head-group) and a
partial out-projection; the host sums the two head-group partials per batch
(the "all-reduce") and adds o_b.

Device schedule (v3, fp8): matmul cost on the PE is (out free size) x
(cycles/row); fp8e4 with MatmulPerfMode.DoubleRow runs 0.5 cycles/row with a
256-deep contraction (2 fp8 weights per PE cell), 4x the fp16 MAC rate.
The error gate (2e-2 rel) rules out fp8 on the value path (V, P, attn-out,
o_w each land ~2.4% e4m3 quantization 1:1 in the final output), so fp8 is
applied only where the error lands as an *absolute* score perturbation:

  - scores (q.k): q and kt stored fp8e4 (sqrt(softmax scale) folded into
    each to stay clear of fp8 subnormals).  DoubleRow with the contraction
    zero-padded 64->128: kt tiles are [128, 2, C] with sub1 = 0 (0 x finite
    junk = 0; NaN would poison, so the q sub1 halves are zeroed too).
    131072 -> 65536 PE cycles.
  - q/k/v projections: both operands come from DRAM, so the host ships
    fp8 value+residual pairs (a = a8 + aR) and the kernel accumulates
    a8@w8 + a8@wR + aR@w8 in one PSUM group - fp16-level accuracy at
    3 x 0.25 = 0.75x the fp16 cost.  Weights are pre-scaled x256 (values
    would sit in fp8 subnormal range); the epilogue tensor_scalar folds
    the 1/256 into its existing multiply, and for V the x256 rides into
    the fp16 V tiles and is cancelled by scaling the softmax-denominator
    ones column x256 (the reciprocal then yields 1/(256*sum_p)).

attn-V keeps the fp16 probabilities-stationary form with the [V_h | c]
65-wide moving operand; out-projection keeps fp16 with PE transposes.

Engine budget per core (cost model): ACT exp 128x[128,1024] ~= 133us is the
wall; PE ~= 122us (was 161), DVE ~= 80us (normalize batched via a stride-0
broadcast tensor_tensor, transpose copies merged to [128,512]).
"""

import sys

sys.path.insert(0, "/opt/trn_rl_repo")

import numpy as np
import ml_dtypes

F8NP = ml_dtypes.float8_e4m3

B, S, E, C, D = 4, 2048, 1024, 1024, 768
H, HD = 16, 64
EL = E // 2          # embed columns per head-group (8 heads)
N_CORES = 8
NS = S // 512        # s-tiles of 512
NEP = E // 256       # Q-proj contraction double-chunks (4)
NDP = D // 256       # K/V-proj contraction double-chunks (3)
CC = C // 128        # c chunks of 128
HP = EL // 128       # head pairs per core (4)
WS = 256.0           # fp8 weight pre-scale (q/k/v projection weights)

_built = None
_last_results = None


def _build(reps=1, nop_us=0):
    import concourse.bacc as bacc
    import concourse.mybir as mybir
    from concourse.tile import TileContext

    F32 = mybir.dt.float32
    F16 = mybir.dt.float16
    F8 = mybir.dt.float8e4
    Exp = mybir.ActivationFunctionType.Exp
    DR = mybir.MatmulPerfMode.DoubleRow

    nc = bacc.Bacc(None, target_bir_lowering=False)

    x8 = nc.declare_dram_parameter("x8", [E, S], F8, isOutput=False)
    xr = nc.declare_dram_parameter("xr", [E, S], F8, isOutput=False)
    ctxp = nc.declare_dram_parameter("ctxp", [D, 2 * C], F8, isOutput=False)
    qwp = nc.declare_dram_parameter("qwp", [128, 2 * NEP * 2 * EL], F8,
                                    isOutput=False)
    kwp = nc.declare_dram_parameter("kwp", [128, 2 * NDP * 2 * EL], F8,
                                    isOutput=False)
    vw8 = nc.declare_dram_parameter("vw8", [128, NDP * 2 * EL], F8,
                                    isOutput=False)
    vwr = nc.declare_dram_parameter("vwr", [128, NDP * 2 * EL], F8,
                                    isOutput=False)
    ow = nc.declare_dram_parameter("ow", [EL, E], F16, isOutput=False)
    kqb = nc.declare_dram_parameter("kqb", [EL, 2], F32, isOutput=False)
    vbones = nc.declare_dram_parameter("vbones", [1, EL + 128], F16,
                                       isOutput=False)
    ident = nc.declare_dram_parameter("ident", [128, 128], F16, isOutput=False)
    out = nc.declare_dram_parameter("out", [S, E], F16, isOutput=True)

    with TileContext(nc) as tc:
        with (
            tc.tile_pool(name="wpool", bufs=1) as wpool,
            tc.tile_pool(name="dpool", bufs=1) as dpool,
            tc.tile_pool(name="xpool", bufs=2) as xpool,
            tc.tile_pool(name="ptpool", bufs=28) as ptpool,
            tc.tile_pool(name="otpool", bufs=4) as otpool,
            tc.tile_pool(name="ttpool", bufs=12) as ttpool,
            tc.tile_pool(name="spool", bufs=2) as spool,
            tc.tile_pool(name="opool", bufs=2) as opool,
            tc.tile_pool(name="pspool", bufs=1, space="PSUM") as pspool,
        ):
          for _rep in range(reps):
            # ---- weight / context tiles (fp8 value + residual merged into
            # single params: each dma_start burns a ~630ns global issue slot,
            # so the value/residual pair rides one DMA) -------------------
            qwp_t = wpool.tile([128, 2 * NEP * 2 * EL], F8, name="qwp_t")
            qwp_v = qwp_t.rearrange("p (t j i m) -> p t j i m", t=2, j=NEP, i=2)
            qw8_v, qwr_v = qwp_v[:, 0], qwp_v[:, 1]
            kwp_t = wpool.tile([128, 2 * NDP * 2 * EL], F8, name="kwp_t")
            kwp_v = kwp_t.rearrange("p (t j i m) -> p t j i m", t=2, j=NDP, i=2)
            kw8_v, kwr_v = kwp_v[:, 0], kwp_v[:, 1]
            vw8_t = wpool.tile([128, NDP * 2 * EL], F8, name="vw8_t")
            vw8_v = vw8_t.rearrange("p (j i m) -> p j i m", j=NDP, i=2)
            vwr_t = wpool.tile([128, NDP * 2 * EL], F8, name="vwr_t")
            vwr_v = vwr_t.rearrange("p (j i m) -> p j i m", j=NDP, i=2)
            # ctx layout [p, col-half, d-chunk, value/resid, 512] so the
            # half-column DMAs balance to 3 dims
            ctxp_t = dpool.tile([128, 2 * 6 * 2 * 512], F8, name="ctxp_t")
            ctxp_v = ctxp_t.rearrange("p (h d t c) -> p h d t c",
                                      h=2, d=6, t=2)
            ctx8h = [ctxp_v[:, 0, :, 0, :], ctxp_v[:, 1, :, 0, :]]
            cxrh = [ctxp_v[:, 0, :, 1, :], ctxp_v[:, 1, :, 1, :]]

            # All DMA transfers serialize on one global lane in the cost
            # model (each dma_start also burns a ~630ns issue slot), so the
            # prologue issue order IS the arrival order.  Order the lane so
            # each consumer's last dependency lands just before it runs:
            #   kwp, x8, qwp[m0], kqb, ctxp[cols 0-511], xr, vw8, vbones,
            #   ctxp[cols 512-1023], vwr, ident, qwp[m1-3], ow
            kqb_t = wpool.tile([128, 2 * HP], F32, name="kqb_t")
            kb_sb = [kqb_t[:, 2 * m:2 * m + 1] for m in range(HP)]
            qb_sb = [kqb_t[:, 2 * m + 1:2 * m + 2] for m in range(HP)]
            qwp_p = qwp.rearrange("p (t j i m) -> p t j i m", t=2, j=NEP, i=2)
            ctxp_p = ctxp.rearrange("(k p) (h t c) -> p k h t c",
                                    p=128, h=2, t=2)
            nc.sync.dma_start(out=kwp_t[:], in_=kwp[:])
            nc.sync.dma_start(out=qwp_v[:, :, :, :, 0:128],
                              in_=qwp_p[:, :, :, :, 0:128])

            # ---- K^T tiles: [hd-pair 128, sub 2, C] fp8, sub1 = 0 ---------
            kt_sb = []
            kt_v = []
            for m in range(HP):
                t = dpool.tile([128, 2 * C], F8, name=f"kt{m}")
                kt_sb.append(t)
                kt_v.append(t.rearrange("p (i c) -> p i c", i=2))

            # ---- static Q^T tiles: [128, parity 2, sub 2, 512] fp8 --------
            q8_t = []
            q8_v = []
            for m in range(HP):
                t = wpool.tile([128, 2 * 2 * 512], F8, name=f"q8_{m}")
                q8_t.append(t)
                q8_v.append(t.rearrange("p (a i s) -> p a i s", a=2, i=2))

            def zmem(m):
                return []   # kt/q8 sub1 now carry kR / duplicated q8

            # ---- V tiles: [C rows, 8 heads x 65] fp16 (values x WS) -------
            v_sb = []
            for mc in range(CC):
                t = dpool.tile([128, 8 * 65], F16, name=f"v{mc}")
                v_sb.append(t)

            # ---- projection matmul thunks (3-pass fp8 DoubleRow) ----------
            def kgroup_thunks(pairs):
                """Per-matmul thunks for K-proj groups (hp, chalf).  The
                epilogue splits: the PSUM-reading descale runs with the
                group (freeing the acc bank for the next group quickly);
                the fp8 quantize + DoubleRow-residual writes into kt are
                appended as separate thunks so the DVE drain doesn't stall
                the PE's acc-slot pipeline."""
                state = {}

                def f(g, i):
                    hp, chalf = pairs[g]
                    if i == 0:
                        state[g] = pspool.tile([128, 512], F32, name="acc_ps",
                                               tag="acc", bufs=2)
                    ps = state[g]
                    pi, j = i // NDP, i % NDP
                    st_v = [kw8_v, kwr_v, kw8_v][pi]
                    mv_v = [ctx8h, ctx8h, cxrh][pi][chalf]
                    nc.tensor.matmul(
                        ps[:],
                        st_v[:, j, :, hp * 128:(hp + 1) * 128],
                        mv_v[:, 2 * j:2 * j + 2, :],
                        start=(i == 0), stop=(i == 8), perf_mode=DR,
                    )
                    if i == 8:
                        kv = spool.tile([128, 512], F32, name="ksc",
                                        tag="ksc", bufs=4)
                        state[(g, 'kv')] = kv
                        nc.vector.tensor_scalar(
                            kv[:], ps[:], 1.0 / WS, kb_sb[hp],
                            mybir.AluOpType.mult, mybir.AluOpType.add,
                        )

                def fin(g, _):
                    hp, chalf = pairs[g]
                    cs = slice(chalf * 512, (chalf + 1) * 512)
                    kv = state.pop((g, 'kv'))
                    nc.vector.tensor_copy(kt_v[hp][:, 0, cs], kv[:])
                    nc.vector.tensor_tensor(
                        kt_v[hp][:, 1, cs], kv[:], kt_v[hp][:, 0, cs],
                        mybir.AluOpType.subtract,
                    )

                out = []
                for g in range(len(pairs)):
                    out.extend((f, g, i) for i in range(9))
                    out.append((fin, g, 0))
                return out

            def vproj_thunks():
                state = {}

                def f(mc, i):
                    if i == 0:
                        state[mc] = pspool.tile([128, 512], F32,
                                                name="acc_ps", tag="acc",
                                                bufs=2)
                    ps = state[mc]
                    pi, j = i // NDP, i % NDP
                    st_v = [ctx8h, ctx8h, cxrh][pi][mc // 4]
                    mv_v = [vw8_v, vwr_v, vw8_v][pi]
                    nc.tensor.matmul(
                        ps[:],
                        st_v[:, 2 * j:2 * j + 2,
                             (mc % 4) * 128:(mc % 4 + 1) * 128],
                        mv_v[:, j, :, :],
                        start=(i == 0), stop=(i == 8), perf_mode=DR,
                    )
                    if i == 8:
                        vv = v_sb[mc].rearrange("p (h u) -> p h u", u=65)
                        nc.vector.tensor_add(
                            vv[:, :, 0:64],
                            ps.rearrange("p (h u) -> p h u", u=64),
                            vb_bc.rearrange("p (h u) -> p h u", u=64),
                        )
                        nc.vector.tensor_scalar(
                            vv[:, :, 64:65],
                            vb_bc[:, 0:8].rearrange("p (h u) -> p h u", u=1),
                            0.0, WS,
                            mybir.AluOpType.mult, mybir.AluOpType.add,
                        )  # denom column = WS (folds V's x WS into recip)

                return [(f, mc, i) for mc in range(CC) for i in range(9)]

            # ---- pipelined main loop over s-tiles of 512 ------------------
            xts_all = {}
            tts_all = {}

            def load_x(n, e8=None, er=None):
                t8 = xpool.tile([128, 8 * 512], F8, name="x8t", tag="x8t")
                tr = xpool.tile([128, 8 * 512], F8, name="xrt", tag="xrt")
                (e8 or nc.sync).dma_start(
                    out=t8.rearrange("p (c w) -> p c w", w=512),
                    in_=x8[:, n * 512:(n + 1) * 512]
                    .rearrange("(c p) w -> p c w", p=128),
                )
                (er or nc.sync).dma_start(
                    out=tr.rearrange("p (c w) -> p c w", w=512),
                    in_=xr[:, n * 512:(n + 1) * 512]
                    .rearrange("(c p) w -> p c w", p=128),
                )
                xts_all[n] = (t8.rearrange("p (c w) -> p c w", w=512),
                              tr.rearrange("p (c w) -> p c w", w=512))

            def qproj_thunks(n):
                """48 DR-matmul thunks computing q8 (fp8) for s-tile n."""
                state = {}
                thunks = []
                par = n % 2

                def f(m, idx):
                    if idx == 0:
                        state[m] = pspool.tile(
                            [128, 512], F32, name="acc_ps", tag="acc", bufs=2)
                    ps = state[m]
                    pi, j = idx // NEP, idx % NEP
                    x8v, xrv = xts_all[n]
                    st_v = [qw8_v, qwr_v, qw8_v][pi]
                    mv = [x8v, x8v, xrv][pi]
                    nc.tensor.matmul(
                        ps[:],
                        st_v[:, j, :, m * 128:(m + 1) * 128],
                        mv[:, 2 * j:2 * j + 2, :],
                        start=(idx == 0), stop=(idx == 3 * NEP - 1),
                        perf_mode=DR,
                    )
                    if idx == 3 * NEP - 1:
                        nc.vector.tensor_scalar(
                            q8_v[m][:, par, 0, :], ps[:],
                            1.0 / WS, qb_sb[m],
                            mybir.AluOpType.mult, mybir.AluOpType.add,
                        )
                        nc.vector.tensor_copy(q8_v[m][:, par, 1, :],
                                              q8_v[m][:, par, 0, :])

                for m in range(HP):
                    for idx in range(3 * NEP):
                        thunks.append((f, m, idx))
                return thunks

            def outproj_thunks(n, copy_on_act=False):
                """32 matmul thunks for the out-projection of s-tile n (fp16).

                Stationary = transposed normalized attention tile
                tts_all[n][hp] slice ([128 hd-pair rows, 128 s cols], fp16);
                moving = ow chunk [128, 512].  The epilogue instance runs its
                PSUM->SBUF copies on the ACT engine (idle after the final
                exp) to keep the tail chain off the DVE."""
                state = {}
                thunks = []

                def f(ss, ne, hp):
                    if hp == 0:
                        state[(ss, ne)] = pspool.tile(
                            [128, 512], F32, name="acc_ps", tag="acc", bufs=2)
                        if ne == 0:
                            state[ss] = opool.tile(
                                [128, 1024], F16, name="o_sb", tag="o")
                    ps = state[(ss, ne)]
                    nc.tensor.matmul(
                        ps[:],
                        tts_all[n][hp][:, ss * 128:(ss + 1) * 128],
                        ow_sb[hp][:, ne * 512:(ne + 1) * 512],
                        start=(hp == 0), stop=(hp == HP - 1),
                    )
                    if hp == HP - 1:
                        o_sb = state[ss]
                        if copy_on_act:
                            nc.scalar.activation(
                                o_sb[:, ne * 512:(ne + 1) * 512], ps[:],
                                mybir.ActivationFunctionType.Copy)
                        else:
                            nc.vector.tensor_copy(
                                o_sb[:, ne * 512:(ne + 1) * 512], ps[:])
                        if ne == 1:   # one merged store per s-chunk row
                            nc.sync.dma_start(
                                out=out[n * 512 + ss * 128:
                                        n * 512 + (ss + 1) * 128, :],
                                in_=o_sb[:],
                            )

                for ss in range(4):
                    for ne in range(2):
                        for hp in range(HP):
                            thunks.append((f, ss, ne, hp))
                return thunks

            def run_thunks(ts):
                for f, *args in ts:
                    f(*args)

            # prologue: x(0)/qw on the DVE and Pool queues, then the zero
            # memsets for hp0; K-proj hp0 + Q-proj(0) m=0 run pre-loop so the
            # first exp fires ~6us in; everything else is paced into tile 0.
            nc.sync.dma_start(
                out=kqb_t.rearrange("p (c w) -> p c w", w=2),
                in_=kqb.rearrange("(c p) w -> p c w", p=128),
            )
            x8t0 = xpool.tile([128, 8 * 512], F8, name="x8t", tag="x8t")
            nc.sync.dma_start(
                out=x8t0.rearrange("p (c w) -> p c w", w=512),
                in_=x8[:, 0:512].rearrange("(c p) w -> p c w", p=128),
            )
            nc.sync.dma_start(out=ctxp_v[:, 0], in_=ctxp_p[:, :, 0])
            xrt0 = xpool.tile([128, 8 * 512], F8, name="xrt", tag="xrt")
            nc.sync.dma_start(
                out=xrt0.rearrange("p (c w) -> p c w", w=512),
                in_=xr[:, 0:512].rearrange("(c p) w -> p c w", p=128),
            )
            xts_all[0] = (x8t0.rearrange("p (c w) -> p c w", w=512),
                          xrt0.rearrange("p (c w) -> p c w", w=512))
            nc.sync.dma_start(out=ctxp_v[:, 1], in_=ctxp_p[:, :, 1])
            nc.sync.dma_start(out=vw8_t[:], in_=vw8[:])
            vbo_t = wpool.tile([1, EL + 128], F16, name="vbo_t")
            nc.sync.dma_start(out=vbo_t[:], in_=vbones[:])
            vb_sb = vbo_t[:, 0:EL]
            ones_sb = vbo_t[:, EL:EL + 128]
            nc.sync.dma_start(out=vwr_t[:], in_=vwr[:])
            ident_sb = wpool.tile([128, 128], F16, name="ident_sb")
            nc.sync.dma_start(out=ident_sb[:], in_=ident[:])
            vb_bc = wpool.tile([128, EL], F32, name="vb_bc")
            nc.sync.dma_start(out=qwp_v[:, :, :, :, 128:512],
                              in_=qwp_p[:, :, :, :, 128:512])
            ow_all = wpool.tile([128, HP * E], F16, name="ow_all")
            ow_sb = [ow_all[:, k * E:(k + 1) * E] for k in range(HP)]
            nc.sync.dma_start(
                out=ow_all.rearrange("p (c w) -> p c w", w=E),
                in_=ow.rearrange("(c p) w -> p c w", p=128),
            )
            qp0 = qproj_thunks(0)
            run_thunks(qp0[:2 * NEP])         # m=0 passes A+B
            run_thunks(kgroup_thunks([(0, 0)]))
            run_thunks(qp0[2 * NEP:3 * NEP])  # m=0 pass C + epilogue
            # vb broadcast for the V epilogue
            vb_ps = pspool.tile([128, 512], F32, name="acc_ps", tag="acc",
                                bufs=2)
            nc.tensor.matmul(vb_ps[:], ones_sb[0:1, :], vb_sb[:],
                             start=True, stop=True)
            nc.vector.tensor_copy(vb_bc[:], vb_ps[:])

            deferred_q = {}
            for n in range(NS):
                if n + 1 < NS:
                    load_x(n + 1)
                # anchors[i] = (step, bg-index that must be emitted by that
                # step).  Step map per tile: stage s scores = 4 steps, attn-V
                # = 4, transposes = 1; scores at stage 2m start at step
                # 12+17*(m-1) and need kt[m]+q8[m]; attn-V(0) (step 8) needs
                # the V tiles; stage-0 cpair2 (step 2) needs kt[0] cols
                # 512-1023.  inject() paces linearly between anchors.
                # attn-V pipeline depth: tile 0 runs 5 stages behind (the
                # K/V/Q-proj prologue work must fit before the V deadline);
                # later tiles 3 (keeps the last tile's tail short).
                # Step maps: scores(2m) start / attn-V(0) emission step.
                depth = 5 if n == 0 else (2 if n == NS - 1 else 3)
                SCST = {5: [0, 8, 16, 28], 3: [0, 8, 20, 37],
                        2: [0, 8, 25, 42]}[depth]
                AV0 = {5: 24, 3: 16, 2: 12}[depth]
                anchors = []
                bg = []
                if n == 0:
                    bg += kgroup_thunks([(0, 1)])
                    anchors.append((2, len(bg)))
                    bg += kgroup_thunks([(1, 0), (1, 1)])
                    bg += qp0[3 * NEP:2 * 3 * NEP]
                    anchors.append((SCST[1], len(bg)))
                    bg += kgroup_thunks([(2, 0), (2, 1)])
                    bg += qp0[2 * 3 * NEP:3 * 3 * NEP]
                    anchors.append((SCST[2], len(bg)))
                    bg += vproj_thunks()
                    anchors.append((AV0, len(bg)))
                    bg += kgroup_thunks([(3, 0), (3, 1)])
                    bg += qp0[3 * 3 * NEP:4 * 3 * NEP]
                    anchors.append((SCST[3], len(bg)))
                else:
                    dq = deferred_q.pop(n)
                    for m in range(1, HP):
                        bg += dq[(m - 1) * 3 * NEP:m * 3 * NEP]
                        anchors.append((SCST[m], len(bg)))
                if n + 1 < NS:
                    qp_next = qproj_thunks(n + 1)
                    bg += qp_next[:3 * NEP]          # m=0 in this tile
                    deferred_q[n + 1] = qp_next[3 * NEP:]
                if n >= 1:
                    bg += outproj_thunks(n - 1)
                anchors.append((62, len(bg)))

                tts_all[n] = [None] * HP
                par = n % 2
                n_steps = HP * (2 * 8 + 1)      # (hp, h2, 4 sc + 4 av) + tr
                step = 0
                bg_done = 0
                prev_anchor = [0, 0]

                def inject():
                    nonlocal bg_done
                    while anchors and step >= anchors[0][0]:
                        prev_anchor[:] = anchors.pop(0)
                    ns_, ni_ = anchors[0] if anchors else (n_steps, len(bg))
                    ps_, pi_ = prev_anchor
                    if step >= ns_:
                        target = ni_
                    else:
                        target = pi_ + (ni_ - pi_) * (step - ps_) // max(
                            1, ns_ - ps_)
                    target = max(target, pi_)
                    while bg_done < target:
                        fb, *args = bg[bg_done]
                        fb(*args)
                        bg_done += 1

                # software pipeline over stages s = hp*2 + h2: scores+exp of
                # stage s overlap the attn-V/normalize of stage s-1, so the
                # attn-V matmuls never wait on a just-issued exp.
                stage_p = {}
                ot_t = {}

                def emit_scores(s):
                    nonlocal step
                    hp, h2 = s // 2, s % 2
                    pts = []
                    for cpair in range(4):
                        sc = pspool.tile([128, 1024], F32, name="sc_ps",
                                         tag="sc", bufs=2)
                        for cc in range(2):
                            c = cpair * 2 + cc
                            nc.tensor.matmul(
                                sc[:, cc * 512:(cc + 1) * 512],
                                kt_v[hp][h2 * 64:(h2 + 1) * 64, :,
                                         c * 128:(c + 1) * 128],
                                q8_v[hp][h2 * 64:(h2 + 1) * 64, par, :, :],
                                start=True, stop=True, perf_mode=DR,
                            )
                        p = ptpool.tile([128, 1024], F16, name="pt",
                                        tag="pt")
                        nc.scalar.activation(p[:], sc[:], Exp,
                                             scale=1.0 / 16.0)
                        pts.append(p)
                        step += 1
                        inject()
                    stage_p[s] = pts

                def emit_av(s):
                    # one PSUM accumulation group per s-chunk, groups strictly
                    # sequential (a later group's start re-marks the whole
                    # bank pending-zero, so groups must not interleave)
                    nonlocal step
                    hp, h2 = s // 2, s % 2
                    if h2 == 0:
                        ot_t[hp] = otpool.tile([128, 512], F16, name="ot",
                                               tag="ot")
                    otv = ot_t[hp].rearrange("p (ss c) -> p ss c", c=128)
                    pts = stage_p.pop(s)
                    av = pspool.tile([128, 4 * 65], F32, name="av_ps",
                                     tag="av", bufs=1)
                    avv = av.rearrange("p (s u) -> p s u", u=65)
                    vv = [v_sb[c].rearrange("p (h u) -> p h u", u=65)
                          [:, hp * 2 + h2, :] for c in range(CC)]
                    for ss in range(4):
                        for c in range(CC):
                            nc.tensor.matmul(
                                avv[:, ss, :],
                                pts[c // 2][:, (c % 2) * 512 + ss * 128:
                                            (c % 2) * 512 + (ss + 1) * 128],
                                vv[c],
                                start=(c == 0), stop=(c == CC - 1),
                            )
                        step += 1
                        inject()
                    rs = spool.tile([128, 4], F32, name="rs", tag="rs")
                    with nc.allow_low_precision("softmax denom"):
                        nc.vector.reciprocal(
                            rs.rearrange("p (s u) -> p s u", u=1),
                            avv[:, :, 64:65])
                    # batched normalize: all 4 s-chunks in one tensor_tensor
                    # via a stride-0 broadcast of the per-chunk reciprocal
                    nc.vector.tensor_tensor(
                        otv[:, :, h2 * 64:(h2 + 1) * 64],
                        avv[:, :, 0:64],
                        rs[:].rearrange("p (s u) -> p s u", u=1)
                        .broadcast_to([128, 4, 64]),
                        mybir.AluOpType.mult,
                    )

                def emit_transposes(hp):
                    # transpose ot -> [128 hd-pair, 128 s] fp16 stationary
                    # for the out-projection; single merged PSUM->SBUF copy
                    nonlocal step
                    step += 1
                    inject()
                    otv = ot_t[hp].rearrange("p (ss c) -> p ss c", c=128)
                    trv = pspool.tile([128, 512], F16, name="tr_ps",
                                      tag="tr", bufs=1)
                    for ss in range(4):
                        nc.tensor.matmul(
                            trv[:, ss * 128:(ss + 1) * 128],
                            otv[:, ss, :], ident_sb[:],
                            is_transpose=True, start=True, stop=True,
                        )
                    tt = ttpool.tile([128, 512], F16, name="tt", tag="tt")
                    nc.vector.tensor_copy(tt[:], trv[:])
                    tts_all[n][hp] = tt

                for s in range(8):
                    emit_scores(s)
                    if s >= depth:
                        emit_av(s - depth)
                        if (s - depth) % 2 == 1:   # both h2 of a pair done
                            emit_transposes((s - depth) // 2)
                for s2 in range(8 - depth, 8):
                    emit_av(s2)
                    if s2 % 2 == 1:
                        emit_transposes(s2 // 2)
                run_thunks(bg[bg_done:])

            # epilogue: out-projection of the last s-tile
            run_thunks(outproj_thunks(NS - 1, copy_on_act=True))

          # timing aid: calibrated delay chain on the otherwise-idle gpsimd
          # engine; kernel exec time = max(real work, nop chain)
          if nop_us:
            NOP_CYC = 48000
            for _ in range(int(nop_us * 1200 / NOP_CYC)):
                nc.gpsimd.nop(cycle_cnt=NOP_CYC, nofuse=True)

    nc.finalize()
    return nc


def _fp8_pair(a):
    """fp8 value + residual pair such that a ~= a8 + ar (float32 in)."""
    a8 = a.astype(F8NP)
    ar = (a - a8.astype(np.float32)).astype(F8NP)
    return a8, ar


def _pack_w(w, npair):
    """[npair*256, EL] -> [128, npair, 2, EL] DoubleRow packing, flattened."""
    return np.ascontiguousarray(
        w.reshape(npair, 2, 128, EL).transpose(2, 0, 1, 3).reshape(128, -1))


def kernel(x, context, q_w, q_b, k_w, k_b, v_w, v_b, o_w, o_b):
    global _built, _last_results
    from concourse.bass_utils import run_bass_kernel_spmd

    if _built is None:
        _built = _build()
    nc = _built

    # sqrt(softmax scale) x4 per side: scores land x16 in PSUM (descaled in
    # the exp's scale arg); keeps fp8 q/k/kR values clear of subnormals
    rscale = np.float32(4.0 / np.sqrt(np.sqrt(HD)))
    x = np.asarray(x, np.float32)
    context = np.asarray(context, np.float32)
    x8s, xrs, c8s, crs = [], [], [], []
    for b in range(B):
        a8, ar = _fp8_pair(np.ascontiguousarray(x[b].T))
        x8s.append(a8)
        xrs.append(ar)
        a8, ar = _fp8_pair(np.ascontiguousarray(context[b].T))
        c8s.append(a8)
        crs.append(ar)

    in_maps = []
    for core in range(N_CORES):
        b, hg = core // 2, core % 2
        el = slice(hg * EL, (hg + 1) * EL)
        qw = np.asarray(q_w, np.float32)[:, el] * rscale * WS
        kw = np.asarray(k_w, np.float32)[:, el] * rscale * WS
        vw = np.asarray(v_w, np.float32)[:, el] * WS
        qw8, qwr = _fp8_pair(qw)
        kw8, kwr = _fp8_pair(kw)
        vw8, vwr = _fp8_pair(vw)
        in_maps.append({
            "x8": x8s[b], "xr": xrs[b],
            "ctxp": np.ascontiguousarray(
                np.stack([c8s[b], crs[b]], axis=1).reshape(D, 2, 2, 512)
                .transpose(0, 2, 1, 3).reshape(D, 2 * C)),
            "qwp": np.concatenate(
                [_pack_w(qw8, NEP), _pack_w(qwr, NEP)], axis=1),
            "kwp": np.concatenate(
                [_pack_w(kw8, NDP), _pack_w(kwr, NDP)], axis=1),
            "vw8": _pack_w(vw8, NDP), "vwr": _pack_w(vwr, NDP),
            "ow": np.ascontiguousarray(
                np.asarray(o_w, np.float32)[el, :]).astype(np.float16),
            "kqb": np.ascontiguousarray(np.stack(
                [np.asarray(k_b, np.float32)[el] * rscale,
                 np.asarray(q_b, np.float32)[el] * rscale], axis=1)),
            "vbones": np.ascontiguousarray(np.concatenate(
                [np.asarray(v_b, np.float32)[el] * WS,
                 np.ones(128, np.float32)])[None, :]).astype(np.float16),
            "ident": np.eye(128, dtype=np.float16),
        })

    res = run_bass_kernel_spmd(nc, in_maps, list(range(N_CORES)))
    _last_results = res

    ob = np.asarray(o_b, np.float32)
    full = np.empty((B, S, E), np.float32)
    for b in range(B):
        full[b] = (res.results[2 * b]["out"].astype(np.float32)
                   + res.results[2 * b + 1]["out"].astype(np.float32) + ob)
    return full


# revision 37
# speedup vs baseline: 1.0022x; 1.0013x over previous
"""Cross-attention kernel for Trainium2, 8 NeuronCores (fp8 DoubleRow rev).

Reference computation (B=4, S=2048, C=1024, E=1024, D=768, H=16, hd=64):
    q = x @ q_w + q_b                 # [B,S,E]
    k = context @ k_w + k_b           # [B,C,E]
    v = context @ v_w + v_b           # [B,C,E]
    attn = softmax(q.k^T / sqrt(hd))  # per head
    out = (attn @ v) @ o_w + o_b      # [B,S,E]

Sharding: 8 cores = 4 batches x 2 head-groups (8 heads = 512 embed cols each).
Each core computes the full attention for its (batch, head-group) and a
partial out-projection; the host sums the two head-group partials per batch
(the "all-reduce") and adds o_b.

Device schedule (v3, fp8): matmul cost on the PE is (out free size) x
(cycles/row); fp8e4 with MatmulPerfMode.DoubleRow runs 0.5 cycles/row with a
256-deep contraction (2 fp8 weights per PE cell), 4x the fp16 MAC rate.
The error gate (2e-2 rel) rules out fp8 on the value path (V, P, attn-out,
o_w each land ~2.4% e4m3 quantization 1:1 in the final output), so fp8 is
applied only where the error lands as an *absolute* score perturbation:

  - scores (q.k): q and kt stored fp8e4 (sqrt(softmax scale) folded into
    each to stay clear of fp8 subnormals).  DoubleRow with the contraction
    zero-padded 64->128: kt tiles are [128, 2, C] with sub1 = 0 (0 x finite
    junk = 0; NaN would poison, so the q sub1 halves are zeroed too).
    131072 -> 65536 PE cycles.
  - q/k/v projections: both operands come from DRAM, so the host ships
    fp8 value+residual pairs (a = a8 + aR) and the kernel accumulates
    a8@w8 + a8@wR + aR@w8 in one PSUM group - fp16-level accuracy at
    3 x 0.25 = 0.75x the fp16 cost.  Weights are pre-scaled x256 (values
    would sit in fp8 subnormal range); the epilogue tensor_scalar folds
    the 1/256 into its existing multiply, and for V the x256 rides into
    the fp16 V tiles and is cancelled by scaling the softmax-denominator
    ones column x256 (the reciprocal then yields 1/(256*sum_p)).

attn-V keeps the fp16 probabilities-stationary form with the [V_h | c]
65-wide moving operand; out-projection keeps fp16 with PE transposes.

Engine budget per core (cost model): ACT exp 128x[128,1024] ~= 133us is the
wall; PE ~= 122us (was 161), DVE ~= 80us (normalize batched via a stride-0
broadcast tensor_tensor, transpose copies merged to [128,512]).
"""

import sys

sys.path.insert(0, "/opt/trn_rl_repo")

import numpy as np
import ml_dtypes

F8NP = ml_dtypes.float8_e4m3

B, S, E, C, D = 4, 2048, 1024, 1024, 768
H, HD = 16, 64
EL = E // 2          # embed columns per head-group (8 heads)
N_CORES = 8
NS = S // 512        # s-tiles of 512
NEP = E // 256       # Q-proj contraction double-chunks (4)
NDP = D // 256       # K/V-proj contraction double-chunks (3)
CC = C // 128        # c chunks of 128
HP = EL // 128       # head pairs per core (4)
WS = 256.0           # fp8 weight pre-scale (q/k/v projection weights)

_built = None
_last_results = None


def _build(reps=1, nop_us=0):
    import concourse.bacc as bacc
    import concourse.mybir as mybir
    from concourse.tile import TileContext

    F32 = mybir.dt.float32
    F16 = mybir.dt.float16
    F8 = mybir.dt.float8e4
    Exp = mybir.ActivationFunctionType.Exp
    DR = mybir.MatmulPerfMode.DoubleRow

    nc = bacc.Bacc(None, target_bir_lowering=False)

    x8 = nc.declare_dram_parameter("x8", [E, S], F8, isOutput=False)
    xr = nc.declare_dram_parameter("xr", [E, S], F8, isOutput=False)
    ctxp = nc.declare_dram_parameter("ctxp", [D, 2 * C], F8, isOutput=False)
    qwp = nc.declare_dram_parameter("qwp", [128, 2 * NEP * 2 * EL], F8,
                                    isOutput=False)
    kwp = nc.declare_dram_parameter("kwp", [128, 2 * NDP * 2 * EL], F8,
                                    isOutput=False)
    vw8 = nc.declare_dram_parameter("vw8", [128, NDP * 2 * EL], F8,
                                    isOutput=False)
    vwr = nc.declare_dram_parameter("vwr", [128, NDP * 2 * EL], F8,
                                    isOutput=False)
    ow = nc.declare_dram_parameter("ow", [EL, E], F16, isOutput=False)
    kqb = nc.declare_dram_parameter("kqb", [EL, 2], F32, isOutput=False)
    vbones = nc.declare_dram_parameter("vbones", [1, EL + 128], F16,
                                       isOutput=False)
    ident = nc.declare_dram_parameter("ident", [128, 128], F16, isOutput=False)
    out = nc.declare_dram_parameter("out", [S, E], F16, isOutput=True)

    with TileContext(nc) as tc:
        with (
            tc.tile_pool(name="wpool", bufs=1) as wpool,
            tc.tile_pool(name="dpool", bufs=1) as dpool,
            tc.tile_pool(name="xpool", bufs=2) as xpool,
            tc.tile_pool(name="ptpool", bufs=28) as ptpool,
            tc.tile_pool(name="otpool", bufs=4) as otpool,
            tc.tile_pool(name="ttpool", bufs=12) as ttpool,
            tc.tile_pool(name="spool", bufs=2) as spool,
            tc.tile_pool(name="opool", bufs=2) as opool,
            tc.tile_pool(name="pspool", bufs=1, space="PSUM") as pspool,
        ):
          for _rep in range(reps):
            # ---- weight / context tiles (fp8 value + residual merged into
            # single params: each dma_start burns a ~630ns global issue slot,
            # so the value/residual pair rides one DMA) -------------------
            qwp_t = wpool.tile([128, 2 * NEP * 2 * EL], F8, name="qwp_t")
            qwp_v = qwp_t.rearrange("p (t j i m) -> p t j i m", t=2, j=NEP, i=2)
            qw8_v, qwr_v = qwp_v[:, 0], qwp_v[:, 1]
            kwp_t = wpool.tile([128, 2 * NDP * 2 * EL], F8, name="kwp_t")
            kwp_v = kwp_t.rearrange("p (t j i m) -> p t j i m", t=2, j=NDP, i=2)
            kw8_v, kwr_v = kwp_v[:, 0], kwp_v[:, 1]
            vw8_t = wpool.tile([128, NDP * 2 * EL], F8, name="vw8_t")
            vw8_v = vw8_t.rearrange("p (j i m) -> p j i m", j=NDP, i=2)
            vwr_t = wpool.tile([128, NDP * 2 * EL], F8, name="vwr_t")
            vwr_v = vwr_t.rearrange("p (j i m) -> p j i m", j=NDP, i=2)
            # ctx layout [p, col-half, d-chunk, value/resid, 512] so the
            # half-column DMAs balance to 3 dims
            ctxp_t = dpool.tile([128, 2 * 6 * 2 * 512], F8, name="ctxp_t")
            ctxp_v = ctxp_t.rearrange("p (h d t c) -> p h d t c",
                                      h=2, d=6, t=2)
            ctx8h = [ctxp_v[:, 0, :, 0, :], ctxp_v[:, 1, :, 0, :]]
            cxrh = [ctxp_v[:, 0, :, 1, :], ctxp_v[:, 1, :, 1, :]]

            # All DMA transfers serialize on one global lane in the cost
            # model (each dma_start also burns a ~630ns issue slot), so the
            # prologue issue order IS the arrival order.  Order the lane so
            # each consumer's last dependency lands just before it runs:
            #   kwp, x8, qwp[m0], kqb, ctxp[cols 0-511], xr, vw8, vbones,
            #   ctxp[cols 512-1023], vwr, ident, qwp[m1-3], ow
            kqb_t = wpool.tile([128, 2 * HP], F32, name="kqb_t")
            kb_sb = [kqb_t[:, 2 * m:2 * m + 1] for m in range(HP)]
            qb_sb = [kqb_t[:, 2 * m + 1:2 * m + 2] for m in range(HP)]
            qwp_p = qwp.rearrange("p (t j i m) -> p t j i m", t=2, j=NEP, i=2)
            ctxp_p = ctxp.rearrange("(k p) (h t c) -> p k h t c",
                                    p=128, h=2, t=2)
            nc.sync.dma_start(out=kwp_t[:], in_=kwp[:])
            nc.sync.dma_start(out=qwp_v[:, :, :, :, 0:128],
                              in_=qwp_p[:, :, :, :, 0:128])

            # ---- K^T tiles: [hd-pair 128, sub 2, C] fp8, sub1 = 0 ---------
            kt_sb = []
            kt_v = []
            for m in range(HP):
                t = dpool.tile([128, 2 * C], F8, name=f"kt{m}")
                kt_sb.append(t)
                kt_v.append(t.rearrange("p (i c) -> p i c", i=2))

            # ---- static Q^T tiles: [128, parity 2, sub 2, 512] fp8 --------
            q8_t = []
            q8_v = []
            for m in range(HP):
                t = wpool.tile([128, 2 * 2 * 512], F8, name=f"q8_{m}")
                q8_t.append(t)
                q8_v.append(t.rearrange("p (a i s) -> p a i s", a=2, i=2))

            def zmem(m):
                return []   # kt/q8 sub1 now carry kR / duplicated q8

            # ---- V tiles: [C rows, 8 heads x 65] fp16 (values x WS) -------
            v_sb = []
            for mc in range(CC):
                t = dpool.tile([128, 8 * 65], F16, name=f"v{mc}")
                v_sb.append(t)

            # ---- projection matmul thunks (3-pass fp8 DoubleRow) ----------
            def kgroup_thunks(pairs):
                """Per-matmul thunks for K-proj groups (hp, chalf).  The
                epilogue splits: the PSUM-reading descale runs with the
                group (freeing the acc bank for the next group quickly);
                the fp8 quantize + DoubleRow-residual writes into kt are
                appended as separate thunks so the DVE drain doesn't stall
                the PE's acc-slot pipeline."""
                state = {}

                def f(g, i):
                    hp, chalf = pairs[g]
                    if i == 0:
                        state[g] = pspool.tile([128, 512], F32, name="acc_ps",
                                               tag="acc", bufs=2)
                    ps = state[g]
                    pi, j = i // NDP, i % NDP
                    st_v = [kw8_v, kwr_v, kw8_v][pi]
                    mv_v = [ctx8h, ctx8h, cxrh][pi][chalf]
                    nc.tensor.matmul(
                        ps[:],
                        st_v[:, j, :, hp * 128:(hp + 1) * 128],
                        mv_v[:, 2 * j:2 * j + 2, :],
                        start=(i == 0), stop=(i == 8), perf_mode=DR,
                    )
                    if i == 8:
                        kv = spool.tile([128, 512], F32, name="ksc",
                                        tag="ksc", bufs=4)
                        state[(g, 'kv')] = kv
                        nc.vector.tensor_scalar(
                            kv[:], ps[:], 1.0 / WS, kb_sb[hp],
                            mybir.AluOpType.mult, mybir.AluOpType.add,
                        )

                def fin(g, _):
                    hp, chalf = pairs[g]
                    cs = slice(chalf * 512, (chalf + 1) * 512)
                    kv = state.pop((g, 'kv'))
                    nc.vector.tensor_copy(kt_v[hp][:, 0, cs], kv[:])
                    nc.vector.tensor_tensor(
                        kt_v[hp][:, 1, cs], kv[:], kt_v[hp][:, 0, cs],
                        mybir.AluOpType.subtract,
                    )

                out = []
                for g in range(len(pairs)):
                    out.extend((f, g, i) for i in range(9))
                    out.append((fin, g, 0))
                return out

            def vproj_thunks():
                state = {}

                def f(mc, i):
                    if i == 0:
                        state[mc] = pspool.tile([128, 512], F32,
                                                name="acc_ps", tag="acc",
                                                bufs=2)
                    ps = state[mc]
                    pi, j = i // NDP, i % NDP
                    st_v = [ctx8h, ctx8h, cxrh][pi][mc // 4]
                    mv_v = [vw8_v, vwr_v, vw8_v][pi]
                    nc.tensor.matmul(
                        ps[:],
                        st_v[:, 2 * j:2 * j + 2,
                             (mc % 4) * 128:(mc % 4 + 1) * 128],
                        mv_v[:, j, :, :],
                        start=(i == 0), stop=(i == 8), perf_mode=DR,
                    )
                    if i == 8:
                        vv = v_sb[mc].rearrange("p (h u) -> p h u", u=65)
                        nc.vector.tensor_add(
                            vv[:, :, 0:64],
                            ps.rearrange("p (h u) -> p h u", u=64),
                            vb_bc.rearrange("p (h u) -> p h u", u=64),
                        )
                        nc.vector.tensor_scalar(
                            vv[:, :, 64:65],
                            vb_bc[:, 0:8].rearrange("p (h u) -> p h u", u=1),
                            0.0, WS,
                            mybir.AluOpType.mult, mybir.AluOpType.add,
                        )  # denom column = WS (folds V's x WS into recip)

                return [(f, mc, i) for mc in range(CC) for i in range(9)]

            # ---- pipelined main loop over s-tiles of 512 ------------------
            xts_all = {}
            tts_all = {}

            def load_x(n, e8=None, er=None):
                t8 = xpool.tile([128, 8 * 512], F8, name="x8t", tag="x8t")
                tr = xpool.tile([128, 8 * 512], F8, name="xrt", tag="xrt")
                (e8 or nc.sync).dma_start(
                    out=t8.rearrange("p (c w) -> p c w", w=512),
                    in_=x8[:, n * 512:(n + 1) * 512]
                    .rearrange("(c p) w -> p c w", p=128),
                )
                (er or nc.sync).dma_start(
                    out=tr.rearrange("p (c w) -> p c w", w=512),
                    in_=xr[:, n * 512:(n + 1) * 512]
                    .rearrange("(c p) w -> p c w", p=128),
                )
                xts_all[n] = (t8.rearrange("p (c w) -> p c w", w=512),
                              tr.rearrange("p (c w) -> p c w", w=512))

            def qproj_thunks(n):
                """48 DR-matmul thunks computing q8 (fp8) for s-tile n."""
                state = {}
                thunks = []
                par = n % 2

                def f(m, idx):
                    if idx == 0:
                        state[m] = pspool.tile(
                            [128, 512], F32, name="acc_ps", tag="acc", bufs=2)
                    ps = state[m]
                    pi, j = idx // NEP, idx % NEP
                    x8v, xrv = xts_all[n]
                    st_v = [qw8_v, qwr_v, qw8_v][pi]
                    mv = [x8v, x8v, xrv][pi]
                    nc.tensor.matmul(
                        ps[:],
                        st_v[:, j, :, m * 128:(m + 1) * 128],
                        mv[:, 2 * j:2 * j + 2, :],
                        start=(idx == 0), stop=(idx == 3 * NEP - 1),
                        perf_mode=DR,
                    )
                    if idx == 3 * NEP - 1:
                        nc.vector.tensor_scalar(
                            q8_v[m][:, par, 0, :], ps[:],
                            1.0 / WS, qb_sb[m],
                            mybir.AluOpType.mult, mybir.AluOpType.add,
                        )
                        nc.vector.tensor_copy(q8_v[m][:, par, 1, :],
                                              q8_v[m][:, par, 0, :])

                for m in range(HP):
                    for idx in range(3 * NEP):
                        thunks.append((f, m, idx))
                return thunks

            def outproj_thunks(n, copy_on_act=False):
                """32 matmul thunks for the out-projection of s-tile n (fp16).

                Stationary = transposed normalized attention tile
                tts_all[n][hp] slice ([128 hd-pair rows, 128 s cols], fp16);
                moving = ow chunk [128, 512].  The epilogue instance runs its
                PSUM->SBUF copies on the ACT engine (idle after the final
                exp) to keep the tail chain off the DVE."""
                state = {}
                thunks = []

                def f(ss, ne, hp):
                    if hp == 0:
                        state[(ss, ne)] = pspool.tile(
                            [128, 512], F32, name="acc_ps", tag="acc", bufs=2)
                        if ne == 0:
                            state[ss] = opool.tile(
                                [128, 1024], F16, name="o_sb", tag="o")
                    ps = state[(ss, ne)]
                    nc.tensor.matmul(
                        ps[:],
                        tts_all[n][hp][:, ss * 128:(ss + 1) * 128],
                        ow_sb[hp][:, ne * 512:(ne + 1) * 512],
                        start=(hp == 0), stop=(hp == HP - 1),
                    )
                    if hp == HP - 1:
                        o_sb = state[ss]
                        if copy_on_act and (2 * ss + ne) % 2 == 0:
                            nc.scalar.activation(
                                o_sb[:, ne * 512:(ne + 1) * 512], ps[:],
                                mybir.ActivationFunctionType.Copy)
                        else:
                            nc.vector.tensor_copy(
                                o_sb[:, ne * 512:(ne + 1) * 512], ps[:])
                        if ne == 1:   # one merged store per s-chunk row
                            nc.sync.dma_start(
                                out=out[n * 512 + ss * 128:
                                        n * 512 + (ss + 1) * 128, :],
                                in_=o_sb[:],
                            )

                for ss in range(4):
                    for ne in range(2):
                        for hp in range(HP):
                            thunks.append((f, ss, ne, hp))
                return thunks

            def run_thunks(ts):
                for f, *args in ts:
                    f(*args)

            # prologue: x(0)/qw on the DVE and Pool queues, then the zero
            # memsets for hp0; K-proj hp0 + Q-proj(0) m=0 run pre-loop so the
            # first exp fires ~6us in; everything else is paced into tile 0.
            nc.sync.dma_start(
                out=kqb_t.rearrange("p (c w) -> p c w", w=2),
                in_=kqb.rearrange("(c p) w -> p c w", p=128),
            )
            x8t0 = xpool.tile([128, 8 * 512], F8, name="x8t", tag="x8t")
            nc.sync.dma_start(
                out=x8t0.rearrange("p (c w) -> p c w", w=512),
                in_=x8[:, 0:512].rearrange("(c p) w -> p c w", p=128),
            )
            nc.sync.dma_start(out=ctxp_v[:, 0], in_=ctxp_p[:, :, 0])
            xrt0 = xpool.tile([128, 8 * 512], F8, name="xrt", tag="xrt")
            nc.sync.dma_start(
                out=xrt0.rearrange("p (c w) -> p c w", w=512),
                in_=xr[:, 0:512].rearrange("(c p) w -> p c w", p=128),
            )
            xts_all[0] = (x8t0.rearrange("p (c w) -> p c w", w=512),
                          xrt0.rearrange("p (c w) -> p c w", w=512))
            nc.sync.dma_start(out=ctxp_v[:, 1], in_=ctxp_p[:, :, 1])
            nc.sync.dma_start(out=vw8_t[:], in_=vw8[:])
            vbo_t = wpool.tile([1, EL + 128], F16, name="vbo_t")
            nc.sync.dma_start(out=vbo_t[:], in_=vbones[:])
            vb_sb = vbo_t[:, 0:EL]
            ones_sb = vbo_t[:, EL:EL + 128]
            nc.sync.dma_start(out=vwr_t[:], in_=vwr[:])
            ident_sb = wpool.tile([128, 128], F16, name="ident_sb")
            nc.sync.dma_start(out=ident_sb[:], in_=ident[:])
            vb_bc = wpool.tile([128, EL], F32, name="vb_bc")
            nc.sync.dma_start(out=qwp_v[:, :, :, :, 128:512],
                              in_=qwp_p[:, :, :, :, 128:512])
            ow_all = wpool.tile([128, HP * E], F16, name="ow_all")
            ow_sb = [ow_all[:, k * E:(k + 1) * E] for k in range(HP)]
            nc.sync.dma_start(
                out=ow_all.rearrange("p (c w) -> p c w", w=E),
                in_=ow.rearrange("(c p) w -> p c w", p=128),
            )
            qp0 = qproj_thunks(0)
            run_thunks(qp0[:2 * NEP])         # m=0 passes A+B
            run_thunks(kgroup_thunks([(0, 0)]))
            run_thunks(qp0[2 * NEP:3 * NEP])  # m=0 pass C + epilogue
            # vb broadcast for the V epilogue
            vb_ps = pspool.tile([128, 512], F32, name="acc_ps", tag="acc",
                                bufs=2)
            nc.tensor.matmul(vb_ps[:], ones_sb[0:1, :], vb_sb[:],
                             start=True, stop=True)
            nc.vector.tensor_copy(vb_bc[:], vb_ps[:])

            deferred_q = {}
            for n in range(NS):
                if n + 1 < NS:
                    load_x(n + 1)
                # anchors[i] = (step, bg-index that must be emitted by that
                # step).  Step map per tile: stage s scores = 4 steps, attn-V
                # = 4, transposes = 1; scores at stage 2m start at step
                # 12+17*(m-1) and need kt[m]+q8[m]; attn-V(0) (step 8) needs
                # the V tiles; stage-0 cpair2 (step 2) needs kt[0] cols
                # 512-1023.  inject() paces linearly between anchors.
                # attn-V pipeline depth: tile 0 runs 5 stages behind (the
                # K/V/Q-proj prologue work must fit before the V deadline);
                # later tiles 3 (keeps the last tile's tail short).
                # Step maps: scores(2m) start / attn-V(0) emission step.
                depth = 5 if n == 0 else (2 if n == NS - 1 else 3)
                SCST = {5: [0, 8, 16, 28], 3: [0, 8, 20, 37],
                        2: [0, 8, 25, 42]}[depth]
                AV0 = {5: 24, 3: 16, 2: 12}[depth]
                anchors = []
                bg = []
                if n == 0:
                    bg += kgroup_thunks([(0, 1)])
                    anchors.append((2, len(bg)))
                    bg += kgroup_thunks([(1, 0), (1, 1)])
                    bg += qp0[3 * NEP:2 * 3 * NEP]
                    anchors.append((SCST[1], len(bg)))
                    bg += kgroup_thunks([(2, 0), (2, 1)])
                    bg += qp0[2 * 3 * NEP:3 * 3 * NEP]
                    anchors.append((SCST[2], len(bg)))
                    bg += vproj_thunks()
                    anchors.append((AV0, len(bg)))
                    bg += kgroup_thunks([(3, 0), (3, 1)])
                    bg += qp0[3 * 3 * NEP:4 * 3 * NEP]
                    anchors.append((SCST[3], len(bg)))
                else:
                    dq = deferred_q.pop(n)
                    for m in range(1, HP):
                        bg += dq[(m - 1) * 3 * NEP:m * 3 * NEP]
                        anchors.append((SCST[m], len(bg)))
                if n + 1 < NS:
                    qp_next = qproj_thunks(n + 1)
                    bg += qp_next[:3 * NEP]          # m=0 in this tile
                    deferred_q[n + 1] = qp_next[3 * NEP:]
                if n >= 1:
                    bg += outproj_thunks(n - 1)
                anchors.append((62, len(bg)))

                tts_all[n] = [None] * HP
                par = n % 2
                n_steps = HP * (2 * 8 + 1)      # (hp, h2, 4 sc + 4 av) + tr
                step = 0
                bg_done = 0
                prev_anchor = [0, 0]

                def inject():
                    nonlocal bg_done
                    while anchors and step >= anchors[0][0]:
                        prev_anchor[:] = anchors.pop(0)
                    ns_, ni_ = anchors[0] if anchors else (n_steps, len(bg))
                    ps_, pi_ = prev_anchor
                    if step >= ns_:
                        target = ni_
                    else:
                        target = pi_ + (ni_ - pi_) * (step - ps_) // max(
                            1, ns_ - ps_)
                    target = max(target, pi_)
                    while bg_done < target:
                        fb, *args = bg[bg_done]
                        fb(*args)
                        bg_done += 1

                # software pipeline over stages s = hp*2 + h2: scores+exp of
                # stage s overlap the attn-V/normalize of stage s-1, so the
                # attn-V matmuls never wait on a just-issued exp.
                stage_p = {}
                ot_t = {}

                def emit_scores(s):
                    nonlocal step
                    hp, h2 = s // 2, s % 2
                    pts = []
                    for cpair in range(4):
                        sc = pspool.tile([128, 1024], F32, name="sc_ps",
                                         tag="sc", bufs=2)
                        for cc in range(2):
                            c = cpair * 2 + cc
                            nc.tensor.matmul(
                                sc[:, cc * 512:(cc + 1) * 512],
                                kt_v[hp][h2 * 64:(h2 + 1) * 64, :,
                                         c * 128:(c + 1) * 128],
                                q8_v[hp][h2 * 64:(h2 + 1) * 64, par, :, :],
                                start=True, stop=True, perf_mode=DR,
                            )
                        p = ptpool.tile([128, 1024], F16, name="pt",
                                        tag="pt")
                        nc.scalar.activation(p[:], sc[:], Exp,
                                             scale=1.0 / 16.0)
                        pts.append(p)
                        step += 1
                        inject()
                    stage_p[s] = pts

                def emit_av(s):
                    # one PSUM accumulation group per s-chunk, groups strictly
                    # sequential (a later group's start re-marks the whole
                    # bank pending-zero, so groups must not interleave)
                    nonlocal step
                    hp, h2 = s // 2, s % 2
                    if h2 == 0:
                        ot_t[hp] = otpool.tile([128, 512], F16, name="ot",
                                               tag="ot")
                    otv = ot_t[hp].rearrange("p (ss c) -> p ss c", c=128)
                    pts = stage_p.pop(s)
                    av = pspool.tile([128, 4 * 65], F32, name="av_ps",
                                     tag="av", bufs=1)
                    avv = av.rearrange("p (s u) -> p s u", u=65)
                    vv = [v_sb[c].rearrange("p (h u) -> p h u", u=65)
                          [:, hp * 2 + h2, :] for c in range(CC)]
                    for ss in range(4):
                        for c in range(CC):
                            nc.tensor.matmul(
                                avv[:, ss, :],
                                pts[c // 2][:, (c % 2) * 512 + ss * 128:
                                            (c % 2) * 512 + (ss + 1) * 128],
                                vv[c],
                                start=(c == 0), stop=(c == CC - 1),
                            )
                        step += 1
                        inject()
                    rs = spool.tile([128, 4], F32, name="rs", tag="rs")
                    with nc.allow_low_precision("softmax denom"):
                        nc.vector.reciprocal(
                            rs.rearrange("p (s u) -> p s u", u=1),
                            avv[:, :, 64:65])
                    # batched normalize: all 4 s-chunks in one tensor_tensor
                    # via a stride-0 broadcast of the per-chunk reciprocal
                    nc.vector.tensor_tensor(
                        otv[:, :, h2 * 64:(h2 + 1) * 64],
                        avv[:, :, 0:64],
                        rs[:].rearrange("p (s u) -> p s u", u=1)
                        .broadcast_to([128, 4, 64]),
                        mybir.AluOpType.mult,
                    )

                def emit_transposes(hp):
                    # transpose ot -> [128 hd-pair, 128 s] fp16 stationary
                    # for the out-projection; single merged PSUM->SBUF copy
                    nonlocal step
                    step += 1
                    inject()
                    otv = ot_t[hp].rearrange("p (ss c) -> p ss c", c=128)
                    trv = pspool.tile([128, 512], F16, name="tr_ps",
                                      tag="tr", bufs=1)
                    for ss in range(4):
                        nc.tensor.matmul(
                            trv[:, ss * 128:(ss + 1) * 128],
                            otv[:, ss, :], ident_sb[:],
                            is_transpose=True, start=True, stop=True,
                        )
                    tt = ttpool.tile([128, 512], F16, name="tt", tag="tt")
                    nc.vector.tensor_copy(tt[:], trv[:])
                    tts_all[n][hp] = tt

                for s in range(8):
                    emit_scores(s)
                    if s >= depth:
                        emit_av(s - depth)
                        if (s - depth) % 2 == 1:   # both h2 of a pair done
                            emit_transposes((s - depth) // 2)
                for s2 in range(8 - depth, 8):
                    if n == NS - 1 and s2 == 7:
                        # dependency-free transposes bridge the PE wait for
                        # the final exps so the p-state clock stays at 2.4GHz
                        # for the epilogue out-projection
                        fil = pspool.tile([128, 512], F16, name="tr_ps",
                                          tag="tr", bufs=1)
                        for _ in range(12):
                            nc.tensor.matmul(
                                fil[:, 0:128], ident_sb[:], ident_sb[:],
                                is_transpose=True, start=True, stop=True,
                            )
                    emit_av(s2)
                    if s2 % 2 == 1:
                        emit_transposes(s2 // 2)
                run_thunks(bg[bg_done:])

            # epilogue: out-projection of the last s-tile
            run_thunks(outproj_thunks(NS - 1, copy_on_act=True))

          # timing aid: calibrated delay chain on the otherwise-idle gpsimd
          # engine; kernel exec time = max(real work, nop chain)
          if nop_us:
            NOP_CYC = 48000
            for _ in range(int(nop_us * 1200 / NOP_CYC)):
                nc.gpsimd.nop(cycle_cnt=NOP_CYC, nofuse=True)

    nc.finalize()
    return nc


def _fp8_pair(a):
    """fp8 value + residual pair such that a ~= a8 + ar (float32 in)."""
    a8 = a.astype(F8NP)
    ar = (a - a8.astype(np.float32)).astype(F8NP)
    return a8, ar


def _pack_w(w, npair):
    """[npair*256, EL] -> [128, npair, 2, EL] DoubleRow packing, flattened."""
    return np.ascontiguousarray(
        w.reshape(npair, 2, 128, EL).transpose(2, 0, 1, 3).reshape(128, -1))


def kernel(x, context, q_w, q_b, k_w, k_b, v_w, v_b, o_w, o_b):
    global _built, _last_results
    from concourse.bass_utils import run_bass_kernel_spmd

    if _built is None:
        _built = _build()
    nc = _built

    # sqrt(softmax scale) x4 per side: scores land x16 in PSUM (descaled in
    # the exp's scale arg); keeps fp8 q/k/kR values clear of subnormals
    rscale = np.float32(4.0 / np.sqrt(np.sqrt(HD)))
    x = np.asarray(x, np.float32)
    context = np.asarray(context, np.float32)
    x8s, xrs, c8s, crs = [], [], [], []
    for b in range(B):
        a8, ar = _fp8_pair(np.ascontiguousarray(x[b].T))
        x8s.append(a8)
        xrs.append(ar)
        a8, ar = _fp8_pair(np.ascontiguousarray(context[b].T))
        c8s.append(a8)
        crs.append(ar)

    in_maps = []
    for core in range(N_CORES):
        b, hg = core // 2, core % 2
        el = slice(hg * EL, (hg + 1) * EL)
        qw = np.asarray(q_w, np.float32)[:, el] * rscale * WS
        kw = np.asarray(k_w, np.float32)[:, el] * rscale * WS
        vw = np.asarray(v_w, np.float32)[:, el] * WS
        qw8, qwr = _fp8_pair(qw)
        kw8, kwr = _fp8_pair(kw)
        vw8, vwr = _fp8_pair(vw)
        in_maps.append({
            "x8": x8s[b], "xr": xrs[b],
            "ctxp": np.ascontiguousarray(
                np.stack([c8s[b], crs[b]], axis=1).reshape(D, 2, 2, 512)
                .transpose(0, 2, 1, 3).reshape(D, 2 * C)),
            "qwp": np.concatenate(
                [_pack_w(qw8, NEP), _pack_w(qwr, NEP)], axis=1),
            "kwp": np.concatenate(
                [_pack_w(kw8, NDP), _pack_w(kwr, NDP)], axis=1),
            "vw8": _pack_w(vw8, NDP), "vwr": _pack_w(vwr, NDP),
            "ow": np.ascontiguousarray(
                np.asarray(o_w, np.float32)[el, :]).astype(np.float16),
            "kqb": np.ascontiguousarray(np.stack(
                [np.asarray(k_b, np.float32)[el] * rscale,
                 np.asarray(q_b, np.float32)[el] * rscale], axis=1)),
            "vbones": np.ascontiguousarray(np.concatenate(
                [np.asarray(v_b, np.float32)[el] * WS,
                 np.ones(128, np.float32)])[None, :]).astype(np.float16),
            "ident": np.eye(128, dtype=np.float16),
        })

    res = run_bass_kernel_spmd(nc, in_maps, list(range(N_CORES)))
    _last_results = res

    ob = np.asarray(o_b, np.float32)
    full = np.empty((B, S, E), np.float32)
    for b in range(B):
        full[b] = (res.results[2 * b]["out"].astype(np.float32)
                   + res.results[2 * b + 1]["out"].astype(np.float32) + ob)
    return full


# revision 38
# speedup vs baseline: 1.0137x; 1.0116x over previous
"""Cross-attention kernel for Trainium2, 8 NeuronCores (fp8 DoubleRow rev).

Reference computation (B=4, S=2048, C=1024, E=1024, D=768, H=16, hd=64):
    q = x @ q_w + q_b                 # [B,S,E]
    k = context @ k_w + k_b           # [B,C,E]
    v = context @ v_w + v_b           # [B,C,E]
    attn = softmax(q.k^T / sqrt(hd))  # per head
    out = (attn @ v) @ o_w + o_b      # [B,S,E]

Sharding: 8 cores = 4 batches x 2 head-groups (8 heads = 512 embed cols each).
Each core computes the full attention for its (batch, head-group) and a
partial out-projection; the host sums the two head-group partials per batch
(the "all-reduce") and adds o_b.

Device schedule (v3, fp8): matmul cost on the PE is (out free size) x
(cycles/row); fp8e4 with MatmulPerfMode.DoubleRow runs 0.5 cycles/row with a
256-deep contraction (2 fp8 weights per PE cell), 4x the fp16 MAC rate.
The error gate (2e-2 rel) rules out fp8 on the value path (V, P, attn-out,
o_w each land ~2.4% e4m3 quantization 1:1 in the final output), so fp8 is
applied only where the error lands as an *absolute* score perturbation:

  - scores (q.k): q and kt stored fp8e4 (sqrt(softmax scale) folded into
    each to stay clear of fp8 subnormals).  DoubleRow with the contraction
    zero-padded 64->128: kt tiles are [128, 2, C] with sub1 = 0 (0 x finite
    junk = 0; NaN would poison, so the q sub1 halves are zeroed too).
    131072 -> 65536 PE cycles.
  - q/k/v projections: both operands come from DRAM, so the host ships
    fp8 value+residual pairs (a = a8 + aR) and the kernel accumulates
    a8@w8 + a8@wR + aR@w8 in one PSUM group - fp16-level accuracy at
    3 x 0.25 = 0.75x the fp16 cost.  Weights are pre-scaled x256 (values
    would sit in fp8 subnormal range); the epilogue tensor_scalar folds
    the 1/256 into its existing multiply, and for V the x256 rides into
    the fp16 V tiles and is cancelled by scaling the softmax-denominator
    ones column x256 (the reciprocal then yields 1/(256*sum_p)).

attn-V keeps the fp16 probabilities-stationary form with the [V_h | c]
65-wide moving operand; out-projection keeps fp16 with PE transposes.

Engine budget per core (cost model): ACT exp 128x[128,1024] ~= 133us is the
wall; PE ~= 122us (was 161), DVE ~= 80us (normalize batched via a stride-0
broadcast tensor_tensor, transpose copies merged to [128,512]).
"""

import sys

sys.path.insert(0, "/opt/trn_rl_repo")

import numpy as np
import ml_dtypes

F8NP = ml_dtypes.float8_e4m3

B, S, E, C, D = 4, 2048, 1024, 1024, 768
H, HD = 16, 64
EL = E // 2          # embed columns per head-group (8 heads)
N_CORES = 8
NS = S // 512        # s-tiles of 512
NEP = E // 256       # Q-proj contraction double-chunks (4)
NDP = D // 256       # K/V-proj contraction double-chunks (3)
CC = C // 128        # c chunks of 128
HP = EL // 128       # head pairs per core (4)
WS = 256.0           # fp8 weight pre-scale (q/k/v projection weights)

_built = None
_last_results = None


def _build(reps=1, nop_us=0):
    import concourse.bacc as bacc
    import concourse.mybir as mybir
    from concourse.tile import TileContext

    F32 = mybir.dt.float32
    F16 = mybir.dt.float16
    F8 = mybir.dt.float8e4
    Exp = mybir.ActivationFunctionType.Exp
    DR = mybir.MatmulPerfMode.DoubleRow

    nc = bacc.Bacc(None, target_bir_lowering=False)

    x8 = nc.declare_dram_parameter("x8", [E, S], F8, isOutput=False)
    xr = nc.declare_dram_parameter("xr", [E, S], F8, isOutput=False)
    ctxp = nc.declare_dram_parameter("ctxp", [D, 2 * C], F8, isOutput=False)
    qwp = nc.declare_dram_parameter("qwp", [128, 2 * NEP * 2 * EL], F8,
                                    isOutput=False)
    kwp = nc.declare_dram_parameter("kwp", [128, 2 * NDP * 2 * EL], F8,
                                    isOutput=False)
    vw8 = nc.declare_dram_parameter("vw8", [128, NDP * 2 * EL], F8,
                                    isOutput=False)
    vwr = nc.declare_dram_parameter("vwr", [128, NDP * 2 * EL], F8,
                                    isOutput=False)
    ow = nc.declare_dram_parameter("ow", [EL, E], F16, isOutput=False)
    kqb = nc.declare_dram_parameter("kqb", [EL, 2], F32, isOutput=False)
    vbones = nc.declare_dram_parameter("vbones", [1, EL + 128], F16,
                                       isOutput=False)
    ident = nc.declare_dram_parameter("ident", [128, 128], F16, isOutput=False)
    out = nc.declare_dram_parameter("out", [S, E], F16, isOutput=True)

    with TileContext(nc) as tc:
        with (
            tc.tile_pool(name="wpool", bufs=1) as wpool,
            tc.tile_pool(name="dpool", bufs=1) as dpool,
            tc.tile_pool(name="xpool", bufs=2) as xpool,
            tc.tile_pool(name="ptpool", bufs=28) as ptpool,
            tc.tile_pool(name="otpool", bufs=4) as otpool,
            tc.tile_pool(name="ttpool", bufs=12) as ttpool,
            tc.tile_pool(name="spool", bufs=2) as spool,
            tc.tile_pool(name="opool", bufs=2) as opool,
            tc.tile_pool(name="pspool", bufs=1, space="PSUM") as pspool,
        ):
          for _rep in range(reps):
            # ---- weight / context tiles (fp8 value + residual merged into
            # single params: each dma_start burns a ~630ns global issue slot,
            # so the value/residual pair rides one DMA) -------------------
            qwp_t = wpool.tile([128, 2 * NEP * 2 * EL], F8, name="qwp_t")
            qwp_v = qwp_t.rearrange("p (t j i m) -> p t j i m", t=2, j=NEP, i=2)
            qw8_v, qwr_v = qwp_v[:, 0], qwp_v[:, 1]
            kwp_t = wpool.tile([128, 2 * NDP * 2 * EL], F8, name="kwp_t")
            kwp_v = kwp_t.rearrange("p (t j i m) -> p t j i m", t=2, j=NDP, i=2)
            kw8_v, kwr_v = kwp_v[:, 0], kwp_v[:, 1]
            vw8_t = wpool.tile([128, NDP * 2 * EL], F8, name="vw8_t")
            vw8_v = vw8_t.rearrange("p (j i m) -> p j i m", j=NDP, i=2)
            vwr_t = wpool.tile([128, NDP * 2 * EL], F8, name="vwr_t")
            vwr_v = vwr_t.rearrange("p (j i m) -> p j i m", j=NDP, i=2)
            # ctx layout [p, col-half, d-chunk, value/resid, 512] so the
            # half-column DMAs balance to 3 dims
            ctxp_t = dpool.tile([128, 2 * 6 * 2 * 512], F8, name="ctxp_t")
            ctxp_v = ctxp_t.rearrange("p (h d t c) -> p h d t c",
                                      h=2, d=6, t=2)
            ctx8h = [ctxp_v[:, 0, :, 0, :], ctxp_v[:, 1, :, 0, :]]
            cxrh = [ctxp_v[:, 0, :, 1, :], ctxp_v[:, 1, :, 1, :]]

            # All DMA transfers serialize on one global lane in the cost
            # model (each dma_start also burns a ~630ns issue slot), so the
            # prologue issue order IS the arrival order.  Order the lane so
            # each consumer's last dependency lands just before it runs:
            #   kwp, x8, qwp[m0], kqb, ctxp[cols 0-511], xr, vw8, vbones,
            #   ctxp[cols 512-1023], vwr, ident, qwp[m1-3], ow
            kqb_t = wpool.tile([128, 2 * HP], F32, name="kqb_t")
            kb_sb = [kqb_t[:, 2 * m:2 * m + 1] for m in range(HP)]
            qb_sb = [kqb_t[:, 2 * m + 1:2 * m + 2] for m in range(HP)]
            qwp_p = qwp.rearrange("p (t j i m) -> p t j i m", t=2, j=NEP, i=2)
            ctxp_p = ctxp.rearrange("(k p) (h t c) -> p k h t c",
                                    p=128, h=2, t=2)
            nc.sync.dma_start(out=kwp_t[:], in_=kwp[:])
            nc.sync.dma_start(out=qwp_v[:, :, :, :, 0:128],
                              in_=qwp_p[:, :, :, :, 0:128])

            # ---- K^T tiles: [hd-pair 128, sub 2, C] fp8, sub1 = 0 ---------
            kt_sb = []
            kt_v = []
            for m in range(HP):
                t = dpool.tile([128, 2 * C], F8, name=f"kt{m}")
                kt_sb.append(t)
                kt_v.append(t.rearrange("p (i c) -> p i c", i=2))

            # ---- static Q^T tiles: [128, parity 2, sub 2, 512] fp8 --------
            q8_t = []
            q8_v = []
            for m in range(HP):
                t = wpool.tile([128, 2 * 2 * 512], F8, name=f"q8_{m}")
                q8_t.append(t)
                q8_v.append(t.rearrange("p (a i s) -> p a i s", a=2, i=2))

            def zmem(m):
                return []   # kt/q8 sub1 now carry kR / duplicated q8

            # ---- V tiles: [C rows, 8 heads x 65] fp16 (values x WS) -------
            v_sb = []
            for mc in range(CC):
                t = dpool.tile([128, 8 * 65], F16, name=f"v{mc}")
                v_sb.append(t)

            # ---- projection matmul thunks (3-pass fp8 DoubleRow) ----------
            def kgroup_thunks(pairs):
                """Per-matmul thunks for K-proj groups (hp, chalf).  The
                epilogue splits: the PSUM-reading descale runs with the
                group (freeing the acc bank for the next group quickly);
                the fp8 quantize + DoubleRow-residual writes into kt are
                appended as separate thunks so the DVE drain doesn't stall
                the PE's acc-slot pipeline."""
                state = {}

                def f(g, i):
                    hp, chalf = pairs[g]
                    if i == 0:
                        state[g] = pspool.tile([128, 512], F32, name="acc_ps",
                                               tag="acc", bufs=2)
                    ps = state[g]
                    pi, j = i // NDP, i % NDP
                    st_v = [kw8_v, kwr_v, kw8_v][pi]
                    mv_v = [ctx8h, ctx8h, cxrh][pi][chalf]
                    nc.tensor.matmul(
                        ps[:],
                        st_v[:, j, :, hp * 128:(hp + 1) * 128],
                        mv_v[:, 2 * j:2 * j + 2, :],
                        start=(i == 0), stop=(i == 8), perf_mode=DR,
                    )
                    if i == 8:
                        kv = spool.tile([128, 512], F32, name="ksc",
                                        tag="ksc", bufs=4)
                        state[(g, 'kv')] = kv
                        nc.vector.tensor_scalar(
                            kv[:], ps[:], 1.0 / WS, kb_sb[hp],
                            mybir.AluOpType.mult, mybir.AluOpType.add,
                        )

                def fin(g, _):
                    hp, chalf = pairs[g]
                    cs = slice(chalf * 512, (chalf + 1) * 512)
                    kv = state.pop((g, 'kv'))
                    nc.vector.tensor_copy(kt_v[hp][:, 0, cs], kv[:])
                    nc.vector.tensor_tensor(
                        kt_v[hp][:, 1, cs], kv[:], kt_v[hp][:, 0, cs],
                        mybir.AluOpType.subtract,
                    )

                out = []
                for g in range(len(pairs)):
                    out.extend((f, g, i) for i in range(9))
                    out.append((fin, g, 0))
                return out

            def vproj_thunks():
                state = {}

                def f(mc, i):
                    if i == 0:
                        state[mc] = pspool.tile([128, 512], F32,
                                                name="acc_ps", tag="acc",
                                                bufs=2)
                    ps = state[mc]
                    pi, j = i // NDP, i % NDP
                    st_v = [ctx8h, ctx8h, cxrh][pi][mc // 4]
                    mv_v = [vw8_v, vwr_v, vw8_v][pi]
                    nc.tensor.matmul(
                        ps[:],
                        st_v[:, 2 * j:2 * j + 2,
                             (mc % 4) * 128:(mc % 4 + 1) * 128],
                        mv_v[:, j, :, :],
                        start=(i == 0), stop=(i == 8), perf_mode=DR,
                    )
                    if i == 8:
                        vv = v_sb[mc].rearrange("p (h u) -> p h u", u=65)
                        nc.vector.tensor_add(
                            vv[:, :, 0:64],
                            ps.rearrange("p (h u) -> p h u", u=64),
                            vb_bc.rearrange("p (h u) -> p h u", u=64),
                        )
                        nc.vector.tensor_scalar(
                            vv[:, :, 64:65],
                            vb_bc[:, 0:8].rearrange("p (h u) -> p h u", u=1),
                            0.0, WS,
                            mybir.AluOpType.mult, mybir.AluOpType.add,
                        )  # denom column = WS (folds V's x WS into recip)

                return [(f, mc, i) for mc in range(CC) for i in range(9)]

            # ---- pipelined main loop over s-tiles of 512 ------------------
            xts_all = {}
            tts_all = {}

            def load_x(n, e8=None, er=None):
                t8 = xpool.tile([128, 8 * 512], F8, name="x8t", tag="x8t")
                tr = xpool.tile([128, 8 * 512], F8, name="xrt", tag="xrt")
                (e8 or nc.sync).dma_start(
                    out=t8.rearrange("p (c w) -> p c w", w=512),
                    in_=x8[:, n * 512:(n + 1) * 512]
                    .rearrange("(c p) w -> p c w", p=128),
                )
                (er or nc.sync).dma_start(
                    out=tr.rearrange("p (c w) -> p c w", w=512),
                    in_=xr[:, n * 512:(n + 1) * 512]
                    .rearrange("(c p) w -> p c w", p=128),
                )
                xts_all[n] = (t8.rearrange("p (c w) -> p c w", w=512),
                              tr.rearrange("p (c w) -> p c w", w=512))

            def qproj_thunks(n):
                """48 DR-matmul thunks computing q8 (fp8) for s-tile n."""
                state = {}
                thunks = []
                par = n % 2

                def f(m, idx):
                    if idx == 0:
                        state[m] = pspool.tile(
                            [128, 512], F32, name="acc_ps", tag="acc", bufs=2)
                    ps = state[m]
                    pi, j = idx // NEP, idx % NEP
                    x8v, xrv = xts_all[n]
                    st_v = [qw8_v, qwr_v, qw8_v][pi]
                    mv = [x8v, x8v, xrv][pi]
                    nc.tensor.matmul(
                        ps[:],
                        st_v[:, j, :, m * 128:(m + 1) * 128],
                        mv[:, 2 * j:2 * j + 2, :],
                        start=(idx == 0), stop=(idx == 3 * NEP - 1),
                        perf_mode=DR,
                    )
                    if idx == 3 * NEP - 1:
                        nc.vector.tensor_scalar(
                            q8_v[m][:, par, 0, :], ps[:],
                            1.0 / WS, qb_sb[m],
                            mybir.AluOpType.mult, mybir.AluOpType.add,
                        )
                        nc.vector.tensor_copy(q8_v[m][:, par, 1, :],
                                              q8_v[m][:, par, 0, :])

                for m in range(HP):
                    for idx in range(3 * NEP):
                        thunks.append((f, m, idx))
                return thunks

            def outproj_thunks(n, copy_on_act=False, deep_psum=False):
                """32 matmul thunks for the out-projection of s-tile n (fp16).

                Stationary = transposed normalized attention tile
                tts_all[n][hp] slice ([128 hd-pair rows, 128 s cols], fp16);
                moving = ow chunk [128, 512].  The epilogue instance runs its
                PSUM->SBUF copies on the ACT engine (idle after the final
                exp) to keep the tail chain off the DVE."""
                state = {}
                thunks = []

                def f(ss, ne, hp):
                    if hp == 0:
                        if deep_psum and (2 * ss + ne) % 2:
                            # the exp stream is over: borrow the dead sc
                            # banks so 4 groups pipeline without copy-waits
                            t = pspool.tile([128, 1024], F32, name="sc_ps",
                                            tag="sc", bufs=2)
                            state[(ss, ne)] = t[:, 0:512]
                        else:
                            state[(ss, ne)] = pspool.tile(
                                [128, 512], F32, name="acc_ps", tag="acc",
                                bufs=2)
                        if ne == 0:
                            state[ss] = opool.tile(
                                [128, 1024], F16, name="o_sb", tag="o")
                    ps = state[(ss, ne)]
                    nc.tensor.matmul(
                        ps[:],
                        tts_all[n][hp][:, ss * 128:(ss + 1) * 128],
                        ow_sb[hp][:, ne * 512:(ne + 1) * 512],
                        start=(hp == 0), stop=(hp == HP - 1),
                    )
                    if hp == HP - 1:
                        o_sb = state[ss]
                        if copy_on_act and (2 * ss + ne) % 2 == 0:
                            nc.scalar.activation(
                                o_sb[:, ne * 512:(ne + 1) * 512], ps[:],
                                mybir.ActivationFunctionType.Copy)
                        else:
                            nc.vector.tensor_copy(
                                o_sb[:, ne * 512:(ne + 1) * 512], ps[:])
                        if ne == 1:   # one merged store per s-chunk row
                            nc.sync.dma_start(
                                out=out[n * 512 + ss * 128:
                                        n * 512 + (ss + 1) * 128, :],
                                in_=o_sb[:],
                            )

                for ss in range(4):
                    for ne in range(2):
                        for hp in range(HP):
                            thunks.append((f, ss, ne, hp))
                return thunks

            def run_thunks(ts):
                for f, *args in ts:
                    f(*args)

            # prologue: x(0)/qw on the DVE and Pool queues, then the zero
            # memsets for hp0; K-proj hp0 + Q-proj(0) m=0 run pre-loop so the
            # first exp fires ~6us in; everything else is paced into tile 0.
            nc.sync.dma_start(
                out=kqb_t.rearrange("p (c w) -> p c w", w=2),
                in_=kqb.rearrange("(c p) w -> p c w", p=128),
            )
            x8t0 = xpool.tile([128, 8 * 512], F8, name="x8t", tag="x8t")
            nc.sync.dma_start(
                out=x8t0.rearrange("p (c w) -> p c w", w=512),
                in_=x8[:, 0:512].rearrange("(c p) w -> p c w", p=128),
            )
            nc.sync.dma_start(out=ctxp_v[:, 0], in_=ctxp_p[:, :, 0])
            xrt0 = xpool.tile([128, 8 * 512], F8, name="xrt", tag="xrt")
            nc.sync.dma_start(
                out=xrt0.rearrange("p (c w) -> p c w", w=512),
                in_=xr[:, 0:512].rearrange("(c p) w -> p c w", p=128),
            )
            xts_all[0] = (x8t0.rearrange("p (c w) -> p c w", w=512),
                          xrt0.rearrange("p (c w) -> p c w", w=512))
            nc.sync.dma_start(out=ctxp_v[:, 1], in_=ctxp_p[:, :, 1])
            nc.sync.dma_start(out=vw8_t[:], in_=vw8[:])
            vbo_t = wpool.tile([1, EL + 128], F16, name="vbo_t")
            nc.sync.dma_start(out=vbo_t[:], in_=vbones[:])
            vb_sb = vbo_t[:, 0:EL]
            ones_sb = vbo_t[:, EL:EL + 128]
            nc.sync.dma_start(out=vwr_t[:], in_=vwr[:])
            ident_sb = wpool.tile([128, 128], F16, name="ident_sb")
            nc.sync.dma_start(out=ident_sb[:], in_=ident[:])
            vb_bc = wpool.tile([128, EL], F32, name="vb_bc")
            nc.sync.dma_start(out=qwp_v[:, :, :, :, 128:512],
                              in_=qwp_p[:, :, :, :, 128:512])
            ow_all = wpool.tile([128, HP * E], F16, name="ow_all")
            ow_sb = [ow_all[:, k * E:(k + 1) * E] for k in range(HP)]
            nc.sync.dma_start(
                out=ow_all.rearrange("p (c w) -> p c w", w=E),
                in_=ow.rearrange("(c p) w -> p c w", p=128),
            )
            qp0 = qproj_thunks(0)
            run_thunks(qp0[:2 * NEP])         # m=0 passes A+B
            run_thunks(kgroup_thunks([(0, 0)]))
            run_thunks(qp0[2 * NEP:3 * NEP])  # m=0 pass C + epilogue
            # vb broadcast for the V epilogue
            vb_ps = pspool.tile([128, 512], F32, name="acc_ps", tag="acc",
                                bufs=2)
            nc.tensor.matmul(vb_ps[:], ones_sb[0:1, :], vb_sb[:],
                             start=True, stop=True)
            nc.vector.tensor_copy(vb_bc[:], vb_ps[:])

            deferred_q = {}
            for n in range(NS):
                if n + 1 < NS:
                    load_x(n + 1)
                # anchors[i] = (step, bg-index that must be emitted by that
                # step).  Step map per tile: stage s scores = 4 steps, attn-V
                # = 4, transposes = 1; scores at stage 2m start at step
                # 12+17*(m-1) and need kt[m]+q8[m]; attn-V(0) (step 8) needs
                # the V tiles; stage-0 cpair2 (step 2) needs kt[0] cols
                # 512-1023.  inject() paces linearly between anchors.
                # attn-V pipeline depth: tile 0 runs 5 stages behind (the
                # K/V/Q-proj prologue work must fit before the V deadline);
                # later tiles 3 (keeps the last tile's tail short).
                # Step maps: scores(2m) start / attn-V(0) emission step.
                depth = 5 if n == 0 else (2 if n == NS - 1 else 3)
                SCST = {5: [0, 8, 16, 28], 3: [0, 8, 20, 37],
                        2: [0, 8, 25, 42]}[depth]
                AV0 = {5: 24, 3: 16, 2: 12}[depth]
                anchors = []
                bg = []
                if n == 0:
                    bg += kgroup_thunks([(0, 1)])
                    anchors.append((2, len(bg)))
                    bg += kgroup_thunks([(1, 0), (1, 1)])
                    bg += qp0[3 * NEP:2 * 3 * NEP]
                    anchors.append((SCST[1], len(bg)))
                    bg += kgroup_thunks([(2, 0), (2, 1)])
                    bg += qp0[2 * 3 * NEP:3 * 3 * NEP]
                    anchors.append((SCST[2], len(bg)))
                    bg += vproj_thunks()
                    anchors.append((AV0, len(bg)))
                    bg += kgroup_thunks([(3, 0), (3, 1)])
                    bg += qp0[3 * 3 * NEP:4 * 3 * NEP]
                    anchors.append((SCST[3], len(bg)))
                else:
                    dq = deferred_q.pop(n)
                    for m in range(1, HP):
                        bg += dq[(m - 1) * 3 * NEP:m * 3 * NEP]
                        anchors.append((SCST[m], len(bg)))
                if n + 1 < NS:
                    qp_next = qproj_thunks(n + 1)
                    bg += qp_next[:3 * NEP]          # m=0 in this tile
                    deferred_q[n + 1] = qp_next[3 * NEP:]
                if n >= 1:
                    bg += outproj_thunks(n - 1)
                anchors.append((62, len(bg)))

                tts_all[n] = [None] * HP
                par = n % 2
                n_steps = HP * (2 * 8 + 1)      # (hp, h2, 4 sc + 4 av) + tr
                step = 0
                bg_done = 0
                prev_anchor = [0, 0]

                def inject():
                    nonlocal bg_done
                    while anchors and step >= anchors[0][0]:
                        prev_anchor[:] = anchors.pop(0)
                    ns_, ni_ = anchors[0] if anchors else (n_steps, len(bg))
                    ps_, pi_ = prev_anchor
                    if step >= ns_:
                        target = ni_
                    else:
                        target = pi_ + (ni_ - pi_) * (step - ps_) // max(
                            1, ns_ - ps_)
                    target = max(target, pi_)
                    while bg_done < target:
                        fb, *args = bg[bg_done]
                        fb(*args)
                        bg_done += 1

                # software pipeline over stages s = hp*2 + h2: scores+exp of
                # stage s overlap the attn-V/normalize of stage s-1, so the
                # attn-V matmuls never wait on a just-issued exp.
                stage_p = {}
                ot_t = {}

                def emit_scores(s):
                    nonlocal step
                    hp, h2 = s // 2, s % 2
                    pts = []
                    for cpair in range(4):
                        sc = pspool.tile([128, 1024], F32, name="sc_ps",
                                         tag="sc", bufs=2)
                        for cc in range(2):
                            c = cpair * 2 + cc
                            nc.tensor.matmul(
                                sc[:, cc * 512:(cc + 1) * 512],
                                kt_v[hp][h2 * 64:(h2 + 1) * 64, :,
                                         c * 128:(c + 1) * 128],
                                q8_v[hp][h2 * 64:(h2 + 1) * 64, par, :, :],
                                start=True, stop=True, perf_mode=DR,
                            )
                        p = ptpool.tile([128, 1024], F16, name="pt",
                                        tag="pt")
                        nc.scalar.activation(p[:], sc[:], Exp,
                                             scale=1.0 / 16.0)
                        pts.append(p)
                        step += 1
                        inject()
                    stage_p[s] = pts

                def emit_av(s):
                    # one PSUM accumulation group per s-chunk, groups strictly
                    # sequential (a later group's start re-marks the whole
                    # bank pending-zero, so groups must not interleave)
                    nonlocal step
                    hp, h2 = s // 2, s % 2
                    if h2 == 0:
                        ot_t[hp] = otpool.tile([128, 512], F16, name="ot",
                                               tag="ot")
                    otv = ot_t[hp].rearrange("p (ss c) -> p ss c", c=128)
                    pts = stage_p.pop(s)
                    av = pspool.tile([128, 4 * 65], F32, name="av_ps",
                                     tag="av", bufs=1)
                    avv = av.rearrange("p (s u) -> p s u", u=65)
                    vv = [v_sb[c].rearrange("p (h u) -> p h u", u=65)
                          [:, hp * 2 + h2, :] for c in range(CC)]
                    for ss in range(4):
                        for c in range(CC):
                            nc.tensor.matmul(
                                avv[:, ss, :],
                                pts[c // 2][:, (c % 2) * 512 + ss * 128:
                                            (c % 2) * 512 + (ss + 1) * 128],
                                vv[c],
                                start=(c == 0), stop=(c == CC - 1),
                            )
                        step += 1
                        inject()
                    rs = spool.tile([128, 4], F32, name="rs", tag="rs")
                    with nc.allow_low_precision("softmax denom"):
                        nc.vector.reciprocal(
                            rs.rearrange("p (s u) -> p s u", u=1),
                            avv[:, :, 64:65])
                    # batched normalize: all 4 s-chunks in one tensor_tensor
                    # via a stride-0 broadcast of the per-chunk reciprocal
                    nc.vector.tensor_tensor(
                        otv[:, :, h2 * 64:(h2 + 1) * 64],
                        avv[:, :, 0:64],
                        rs[:].rearrange("p (s u) -> p s u", u=1)
                        .broadcast_to([128, 4, 64]),
                        mybir.AluOpType.mult,
                    )

                def emit_transposes(hp):
                    # transpose ot -> [128 hd-pair, 128 s] fp16 stationary
                    # for the out-projection; single merged PSUM->SBUF copy
                    nonlocal step
                    step += 1
                    inject()
                    otv = ot_t[hp].rearrange("p (ss c) -> p ss c", c=128)
                    trv = pspool.tile([128, 512], F16, name="tr_ps",
                                      tag="tr", bufs=1)
                    for ss in range(4):
                        nc.tensor.matmul(
                            trv[:, ss * 128:(ss + 1) * 128],
                            otv[:, ss, :], ident_sb[:],
                            is_transpose=True, start=True, stop=True,
                        )
                    tt = ttpool.tile([128, 512], F16, name="tt", tag="tt")
                    nc.vector.tensor_copy(tt[:], trv[:])
                    tts_all[n][hp] = tt

                for s in range(8):
                    emit_scores(s)
                    if s >= depth:
                        emit_av(s - depth)
                        if (s - depth) % 2 == 1:   # both h2 of a pair done
                            emit_transposes((s - depth) // 2)
                for s2 in range(8 - depth, 8):
                    if n == NS - 1 and s2 == 7:
                        # dependency-free transposes bridge the PE wait for
                        # the final exps so the p-state clock stays at 2.4GHz
                        # for the epilogue out-projection
                        fil = pspool.tile([128, 512], F16, name="tr_ps",
                                          tag="tr", bufs=1)
                        for _ in range(12):
                            nc.tensor.matmul(
                                fil[:, 0:128], ident_sb[:], ident_sb[:],
                                is_transpose=True, start=True, stop=True,
                            )
                    emit_av(s2)
                    if s2 % 2 == 1:
                        emit_transposes(s2 // 2)
                run_thunks(bg[bg_done:])

            # epilogue: out-projection of the last s-tile
            run_thunks(outproj_thunks(NS - 1, copy_on_act=True,
                                      deep_psum=True))

          # timing aid: calibrated delay chain on the otherwise-idle gpsimd
          # engine; kernel exec time = max(real work, nop chain)
          if nop_us:
            NOP_CYC = 48000
            for _ in range(int(nop_us * 1200 / NOP_CYC)):
                nc.gpsimd.nop(cycle_cnt=NOP_CYC, nofuse=True)

    nc.finalize()
    return nc


def _fp8_pair(a):
    """fp8 value + residual pair such that a ~= a8 + ar (float32 in)."""
    a8 = a.astype(F8NP)
    ar = (a - a8.astype(np.float32)).astype(F8NP)
    return a8, ar


def _pack_w(w, npair):
    """[npair*256, EL] -> [128, npair, 2, EL] DoubleRow packing, flattened."""
    return np.ascontiguousarray(
        w.reshape(npair, 2, 128, EL).transpose(2, 0, 1, 3).reshape(128, -1))


def kernel(x, context, q_w, q_b, k_w, k_b, v_w, v_b, o_w, o_b):
    global _built, _last_results
    from concourse.bass_utils import run_bass_kernel_spmd

    if _built is None:
        _built = _build()
    nc = _built

    # sqrt(softmax scale) x4 per side: scores land x16 in PSUM (descaled in
    # the exp's scale arg); keeps fp8 q/k/kR values clear of subnormals
    rscale = np.float32(4.0 / np.sqrt(np.sqrt(HD)))
    x = np.asarray(x, np.float32)
    context = np.asarray(context, np.float32)
    x8s, xrs, c8s, crs = [], [], [], []
    for b in range(B):
        a8, ar = _fp8_pair(np.ascontiguousarray(x[b].T))
        x8s.append(a8)
        xrs.append(ar)
        a8, ar = _fp8_pair(np.ascontiguousarray(context[b].T))
        c8s.append(a8)
        crs.append(ar)

    in_maps = []
    for core in range(N_CORES):
        b, hg = core // 2, core % 2
        el = slice(hg * EL, (hg + 1) * EL)
        qw = np.asarray(q_w, np.float32)[:, el] * rscale * WS
        kw = np.asarray(k_w, np.float32)[:, el] * rscale * WS
        vw = np.asarray(v_w, np.float32)[:, el] * WS
        qw8, qwr = _fp8_pair(qw)
        kw8, kwr = _fp8_pair(kw)
        vw8, vwr = _fp8_pair(vw)
        in_maps.append({
            "x8": x8s[b], "xr": xrs[b],
            "ctxp": np.ascontiguousarray(
                np.stack([c8s[b], crs[b]], axis=1).reshape(D, 2, 2, 512)
                .transpose(0, 2, 1, 3).reshape(D, 2 * C)),
            "qwp": np.concatenate(
                [_pack_w(qw8, NEP), _pack_w(qwr, NEP)], axis=1),
            "kwp": np.concatenate(
                [_pack_w(kw8, NDP), _pack_w(kwr, NDP)], axis=1),
            "vw8": _pack_w(vw8, NDP), "vwr": _pack_w(vwr, NDP),
            "ow": np.ascontiguousarray(
                np.asarray(o_w, np.float32)[el, :]).astype(np.float16),
            "kqb": np.ascontiguousarray(np.stack(
                [np.asarray(k_b, np.float32)[el] * rscale,
                 np.asarray(q_b, np.float32)[el] * rscale], axis=1)),
            "vbones": np.ascontiguousarray(np.concatenate(
                [np.asarray(v_b, np.float32)[el] * WS,
                 np.ones(128, np.float32)])[None, :]).astype(np.float16),
            "ident": np.eye(128, dtype=np.float16),
        })

    res = run_bass_kernel_spmd(nc, in_maps, list(range(N_CORES)))
    _last_results = res

    ob = np.asarray(o_b, np.float32)
    full = np.empty((B, S, E), np.float32)
    for b in range(B):
        full[b] = (res.results[2 * b]["out"].astype(np.float32)
                   + res.results[2 * b + 1]["out"].astype(np.float32) + ob)
    return full
